# revision 1
# baseline (speedup 1.0000x reference)
"""GCN discriminator kernel for Trainium2 (8 NeuronCores, SPMD).

Math (matching the reference):
  deg[n]  = sum_{e: dst=n} w_e + 1
  dinv    = 1/sqrt(deg)
  norm_e  = dinv[src]*w_e*dinv[dst];  self-loop n: dinv[n]^2
  agg     = sum over incoming edges of norm_e * x[src]         [N, 128]
  h       = leaky_relu(agg @ W1 + b1)                          [N, 256]
  pooled  = segment_mean(h, batch)                             [64, 256]
  z       = leaky_relu(concat(pooled, emb[cls]) @ W2 + b2)
  out     = z @ W3 + b3                                        [64, 1]

Sharding: destinations are range-partitioned across the 8 cores.  Each core
aggregates its ~6.6k destination rows.  The irregular gather x[src] runs as
HBM dma_gather (SWDGE descriptors); the segment-sum runs on the PE as
one-hot (weight-scaled) matmuls; pooling is another one-hot matmul
accumulated in PSUM; pooled sums are AllReduce'd and the tiny MLP runs
redundantly on every core.
"""

import numpy as np
import ml_dtypes
from collections import defaultdict

# ----------------------------------------------------------------- config
CFG = dict(
    N=50000, F=128, HID=256, G=64, NCLS=10,
    NCORES=8,
    XLO=32768,            # rows in the "lo" x tensor (int16-indexable)
    ST_D=128,             # dsts per supertile
    WIN_D=32,             # dsts per PSUM window (matmul M)
    K=128,                # edge slots per chunk (matmul K)
    GRAN=4,               # supertiles per gather granule
    NEG=0.2,
    NO_CC=0,              # debug: skip collective (partial sums only)
    NO_GATHER=0,          # debug: memset instead of dma_gather
    GDT="f32",            # gather dtype: "f32" | "bf16"  (x rows + S weights)
    MMDT="bf16",          # downstream matmul dtype
)


def _np_dt(s):
    return {"f32": np.float32, "bf16": ml_dtypes.bfloat16}[s]


# ================================================================= host prep
class Prep:
    pass


def host_prep(inputs, cfg):
    """Integer/layout preprocessing + normalization weights.

    Returns per-core in_maps plus the static (core-independent) program
    structure.
    """
    N, F, G = cfg["N"], cfg["F"], cfg["G"]
    NC, XLO = cfg["NCORES"], cfg["XLO"]
    ST_D, WIN_D, K, GRAN = cfg["ST_D"], cfg["WIN_D"], cfg["K"], cfg["GRAN"]

    x = np.asarray(inputs["x"], np.float32)
    ei = np.asarray(inputs["edge_index"], np.int64)
    ew = np.asarray(inputs["edge_weight"], np.float32)
    batch = np.asarray(inputs["batch"], np.int64)
    cls = np.asarray(inputs["class_labels"], np.int64)
    W1 = np.asarray(inputs["W1"], np.float32)
    b1 = np.asarray(inputs["b1"], np.float32)
    emb = np.asarray(inputs["emb"], np.float32)
    W2 = np.asarray(inputs["W2"], np.float32)
    b2 = np.asarray(inputs["b2"], np.float32)
    W3 = np.asarray(inputs["W3"], np.float32)
    b3 = np.asarray(inputs["b3"], np.float32)

    HID = W1.shape[1]
    EH = emb.shape[1]

    # --- normalization weights (scalar preprocessing, O(E)) -------------
    row = ei[0]
    col = ei[1]
    deg = np.zeros(N, np.float64)
    np.add.at(deg, col, ew.astype(np.float64))
    deg += 1.0
    dinv = 1.0 / np.sqrt(deg)
    wnorm = (dinv[row] * ew.astype(np.float64) * dinv[col]).astype(np.float32)
    loop = np.arange(N, dtype=np.int64)
    a_src = np.concatenate([row, loop])
    a_dst = np.concatenate([col, loop])
    a_w = np.concatenate([wnorm, (dinv * dinv).astype(np.float32)])

    D = -(-N // NC)          # dsts per core
    NST = -(-D // ST_D)      # supertiles per core
    NWIN = NST * 4

    # --- bucket edges into (core, st, w, half) ---------------------------
    core_of = a_dst // D
    dst_loc = a_dst - core_of * D
    st_of = dst_loc // ST_D
    j_of = dst_loc % WIN_D
    w_of = (dst_loc % ST_D) // WIN_D
    half_of = (a_src >= XLO).astype(np.int64)
    srcl = np.where(half_of == 1, a_src - XLO, a_src)

    # windows flat id per core: st*4+w ; sort by (core, win, half)
    key = ((core_of * NST + st_of) * 4 + w_of) * 2 + half_of
    order = np.argsort(key, kind="stable")
    key_s = key[order]
    srcl_s = srcl[order].astype(np.int32)
    j_s = j_of[order].astype(np.int32)
    w_s = a_w[order]

    nbuckets = NC * NWIN * 2
    cnt = np.bincount(key_s, minlength=nbuckets).reshape(NC, NWIN, 2)
    starts = np.zeros(nbuckets + 1, np.int64)
    np.cumsum(cnt.reshape(-1), out=starts[1:])

    # static chunk counts per (window, half): max over cores, lo >= 1
    ch = -(-cnt // K)                      # ceil
    CH = ch.max(axis=0)                    # [NWIN, 2]
    CH[:, 0] = np.maximum(CH[:, 0], 1)

    # --- static program structure ---------------------------------------
    grans = []
    st = 0
    while st < NST:
        n = min(GRAN, NST - st)
        grans.append((st, n))
        st += n

    # per-granule: chunk order = lo chunks (st,w,k) then hi chunks
    # per-granule MM order = for st: for w: lo ks then hi ks
    gmeta = []
    mm_total = 0
    col_total = 0
    for (st0, nst) in grans:
        wins = [(st, w) for st in range(st0, st0 + nst) for w in range(4)]
        clo = int(sum(CH[st * 4 + w, 0] for st, w in wins))
        chi = int(sum(CH[st * 4 + w, 1] for st, w in wins))
        # chunk index map: (st, w, half, k) -> ci within granule
        cimap = {}
        ci = 0
        for half in (0, 1):
            for (st, w) in wins:
                for k in range(int(CH[st * 4 + w, half])):
                    cimap[(st, w, half, k)] = ci
                    ci += 1
        # MM list: (st, w, half, k) in program order, with mm index
        mms = []
        for (st, w) in wins:
            for half in (0, 1):
                for k in range(int(CH[st * 4 + w, half])):
                    mms.append((st, w, half, k))
        gmeta.append(dict(st0=st0, nst=nst, clo=clo, chi=chi,
                          cimap=cimap, mms=mms,
                          mm_off=mm_total, col_off=col_total))
        mm_total += len(mms)
        col_total += (clo + chi) * (K // 16)
    NMM = mm_total
    NCOLS = col_total

    static = dict(cfg=cfg, D=D, NST=NST, CH=CH, grans=grans, gmeta=gmeta,
                  NMM=NMM, NCOLS=NCOLS, HID=HID, EH=EH)

    # --- per-core tensors ------------------------------------------------
    gdt = _np_dt(cfg["GDT"])
    mmdt = _np_dt(cfg["MMDT"])

    counts = np.maximum(np.bincount(batch, minlength=G), 1).astype(np.float32)
    clt = np.zeros((cfg["NCLS"], G), mmdt)
    clt[cls, np.arange(G)] = 1.0

    w2blk = np.zeros((128, 6 * 128), np.float32)
    for kk in range(3):
        for jj in range(2):
            w2blk[:, (kk * 2 + jj) * 128:(kk * 2 + jj + 1) * 128] = \
                W2[kk * 128:(kk + 1) * 128, jj * 128:(jj + 1) * 128]
    w3m = np.zeros((128, 2), np.float32)
    w3m[:, 0] = W3[0:128, 0]
    w3m[:, 1] = W3[128:256, 0]

    xlo = np.ascontiguousarray(x[:XLO]).astype(gdt)
    xhi = np.ascontiguousarray(x[XLO:]).astype(gdt)

    in_maps = []
    for c in range(NC):
        gidx = np.zeros((NCOLS * 16,), np.int16)
        smat = np.zeros((128, NMM * WIN_D), gdt)
        for gi, gm in enumerate(gmeta):
            # slot list for this granule, chunk order
            nch = gm["clo"] + gm["chi"]
            slots_src = np.zeros((nch * K,), np.int32)
            for (stw_half_k, ci) in gm["cimap"].items():
                st, w, half, k = stw_half_k
                b = (c * NWIN + st * 4 + w) * 2 + half
                s0, s1 = starts[b], starts[b + 1]
                seg = srcl_s[s0:s1]
                part = seg[k * K:(k + 1) * K]
                slots_src[ci * K: ci * K + len(part)] = part
            wrapped = slots_src.reshape(-1, 16).T  # [16, nch*8]
            c0 = gm["col_off"]
            gidx.reshape(-1, 16 * (K // 16))  # noop, keep flat
            # place wrapped into flat gidx at columns [c0, c0+nch*8)
            gidx_view = gidx.reshape(NCOLS, 16)
            gidx_view[c0:c0 + nch * (K // 16), :] = wrapped.T
            # S matrices
            for mi, (st, w, half, k) in enumerate(gm["mms"]):
                b = (c * NWIN + st * 4 + w) * 2 + half
                s0, s1 = starts[b], starts[b + 1]
                jj = j_s[s0:s1][k * K:(k + 1) * K]
                ww = w_s[s0:s1][k * K:(k + 1) * K]
                mm = gm["mm_off"] + mi
                np_rows = np.arange(len(jj))
                smat[np_rows, mm * WIN_D + jj] = ww.astype(gdt)
        # idx tensor [128, NCOLS] replicated over 16-partition groups
        gidx2 = np.tile(gidx.reshape(NCOLS, 16).T, (8, 1))

        # pooling matrices [128, NST*64]
        pmat = np.zeros((128, NST * G), mmdt)
        base = c * D
        for stn in range(NST):
            for p in range(ST_D):
                dg = base + stn * ST_D + p
                if dg < min((c + 1) * D, N):
                    pmat[p, stn * G + batch[dg]] = 1.0

        m = dict(
            xlo=xlo, xhi=xhi,
            gidx=np.ascontiguousarray(gidx2),
            smat=smat,
            pmat=pmat,
            w1=W1.astype(mmdt),
            w2blk=w2blk.astype(mmdt),
            w3=w3m.astype(mmdt),
            b1=b1.reshape(1, HID).astype(mmdt),
            b2=b2.reshape(1, HID).astype(mmdt),
            b3=b3.reshape(1, 1).astype(mmdt),
            embh=emb.astype(mmdt),
            clt=clt,
            counts=counts.reshape(G, 1),
        )
        in_maps.append(m)

    prep = Prep()
    prep.static = static
    prep.in_maps = in_maps
    return prep


# ================================================================= builder
def build(static):
    import concourse.bass as bass
    from concourse import bacc, tile
    import concourse.mybir as mybir

    cfg = static["cfg"]
    N, F, G = cfg["N"], cfg["F"], cfg["G"]
    XLO = cfg["XLO"]
    ST_D, WIN_D, K = cfg["ST_D"], cfg["WIN_D"], cfg["K"]
    NST, CH, gmeta = static["NST"], static["CH"], static["gmeta"]
    NMM, NCOLS = static["NMM"], static["NCOLS"]
    HID, EH = static["HID"], static["EH"]
    NCLS = cfg["NCLS"]
    NEG = cfg["NEG"]
    NC = cfg["NCORES"]

    gdt = {"f32": mybir.dt.float32, "bf16": mybir.dt.bfloat16}[cfg["GDT"]]
    mmdt = {"f32": mybir.dt.float32, "bf16": mybir.dt.bfloat16}[cfg["MMDT"]]
    f32 = mybir.dt.float32
    AF = mybir.ActivationFunctionType

    nc = bacc.Bacc(None, target_bir_lowering=False, debug=True)

    xlo_d = nc.declare_dram_parameter("xlo", [XLO, F], gdt, isOutput=False)
    xhi_d = nc.declare_dram_parameter("xhi", [N - XLO, F], gdt, isOutput=False)
    gidx_d = nc.declare_dram_parameter("gidx", [128, NCOLS], mybir.dt.int16, isOutput=False)
    smat_d = nc.declare_dram_parameter("smat", [128, NMM * WIN_D], gdt, isOutput=False)
    pmat_d = nc.declare_dram_parameter("pmat", [128, NST * G], mmdt, isOutput=False)
    w1_d = nc.declare_dram_parameter("w1", [F, HID], mmdt, isOutput=False)
    w2_d = nc.declare_dram_parameter("w2blk", [128, 6 * 128], mmdt, isOutput=False)
    w3_d = nc.declare_dram_parameter("w3", [128, 2], mmdt, isOutput=False)
    b1_d = nc.declare_dram_parameter("b1", [1, HID], mmdt, isOutput=False)
    b2_d = nc.declare_dram_parameter("b2", [1, HID], mmdt, isOutput=False)
    b3_d = nc.declare_dram_parameter("b3", [1, 1], mmdt, isOutput=False)
    emb_d = nc.declare_dram_parameter("embh", [NCLS, EH], mmdt, isOutput=False)
    clt_d = nc.declare_dram_parameter("clt", [NCLS, G], mmdt, isOutput=False)
    cnt_d = nc.declare_dram_parameter("counts", [G, 1], f32, isOutput=False)
    out_d = nc.declare_dram_parameter("out", [1, G], f32, isOutput=True)

    iden_np = np.eye(128, dtype=_np_dt(cfg["MMDT"]))
    iden_d = nc.inline_tensor(iden_np, name="iden")

    with tile.TileContext(nc) as tc:
        with (
            tc.tile_pool(name="const", bufs=1) as constp,
            tc.tile_pool(name="gat", bufs=2) as gatp,
            tc.tile_pool(name="smp", bufs=2) as smp,
            tc.tile_pool(name="work", bufs=3) as workp,
            tc.tile_pool(name="ps_agg", bufs=2, space="PSUM") as ps_agg,
            tc.tile_pool(name="ps_t", bufs=2, space="PSUM") as ps_t,
            tc.tile_pool(name="ps_h", bufs=2, space="PSUM") as ps_h,
            tc.tile_pool(name="ps_pool", bufs=1, space="PSUM") as ps_pool,
            tc.tile_pool(name="dram", bufs=1, space="DRAM") as dramp,
        ):
            # ---- persistent SBUF loads
            gidx_sb = constp.tile([128, NCOLS], mybir.dt.int16)
            nc.sync.dma_start(out=gidx_sb[:, :], in_=gidx_d[:, :])
            pmat_sb = constp.tile([128, NST * G], mmdt)
            nc.sync.dma_start(out=pmat_sb[:, :], in_=pmat_d[:, :])
            w1_sb = constp.tile([F, HID], mmdt)
            nc.sync.dma_start(out=w1_sb[:, :], in_=w1_d[:, :])
            iden_sb = constp.tile([128, 128], mmdt)
            nc.sync.dma_start(out=iden_sb[:, :], in_=iden_d[:, :])
            b1_sb = constp.tile([1, HID], mmdt)
            nc.sync.dma_start(out=b1_sb[:, :], in_=b1_d[:, :])
            ones_sb = constp.tile([1, 128], mmdt)
            nc.vector.memset(ones_sb[:, :], 1.0)

            pooled_ps = ps_pool.tile([G, HID], f32)

            # ---------------- main loop over granules
            for gi, gm in enumerate(gmeta):
                st0, nst = gm["st0"], gm["nst"]
                clo, chi = gm["clo"], gm["chi"]
                nch = clo + chi
                gt = gatp.tile([128, nch, F], gdt, tag="gt")
                c0 = gm["col_off"]
                if cfg.get("NO_GATHER"):
                    nc.vector.memset(gt[:, :, :], 0.125)
                else:
                    if clo > 0:
                        nc.gpsimd.dma_gather(
                            gt[:, 0:clo, :], xlo_d[:, :],
                            gidx_sb[:, c0:c0 + clo * (K // 16)],
                            num_idxs=clo * K, num_idxs_reg=clo * K,
                            elem_size=F, single_packet=False)
                    if chi > 0:
                        nc.gpsimd.dma_gather(
                            gt[:, clo:nch, :], xhi_d[:, :],
                            gidx_sb[:, c0 + clo * (K // 16):c0 + nch * (K // 16)],
                            num_idxs=chi * K, num_idxs_reg=chi * K,
                            elem_size=F, single_packet=False)

                nmm_g = len(gm["mms"])
                sm_sb = smp.tile([128, nmm_g * WIN_D], gdt, tag="sm")
                m0 = gm["mm_off"]
                nc.sync.dma_start(
                    out=sm_sb[:, :],
                    in_=smat_d[:, m0 * WIN_D:(m0 + nmm_g) * WIN_D])

                # group MMs by supertile
                by_st = defaultdict(list)
                for mi, (st, w, half, k) in enumerate(gm["mms"]):
                    by_st[st].append((mi, w, half, k))

                for st in range(st0, st0 + nst):
                    agg = ps_agg.tile([128, F], f32, tag="agg")
                    # first/last mm per window for start/stop flags
                    win_mms = defaultdict(list)
                    for (mi, w, half, k) in by_st[st]:
                        win_mms[w].append((mi, half, k))
                    for w, lst in sorted(win_mms.items()):
                        for i, (mi, half, k) in enumerate(lst):
                            ci = gm["cimap"][(st, w, half, k)]
                            nc.tensor.matmul(
                                agg[w * WIN_D:(w + 1) * WIN_D, :],
                                lhsT=sm_sb[:, mi * WIN_D:(mi + 1) * WIN_D],
                                rhs=gt[:, ci, :],
                                start=(i == 0), stop=(i == len(lst) - 1),
                                tile_position=(0, w * WIN_D))
                    # evac agg -> sbuf (cast to mm dtype)
                    agg_sb = workp.tile([128, F], mmdt, tag="agg_sb")
                    nc.vector.tensor_copy(out=agg_sb[:, :], in_=agg[:, :])
                    # transpose on PE
                    aggT = ps_t.tile([128, 128], mmdt, tag="aggT")
                    nc.tensor.transpose(aggT[:, :], agg_sb[:, :], iden_sb[:, :])
                    aggT_sb = workp.tile([128, 128], mmdt, tag="aggT_sb")
                    nc.scalar.copy(out=aggT_sb[:, :], in_=aggT[:, :])
                    # W1 + b1
                    h_ps = ps_h.tile([128, HID], f32, tag="h")
                    nc.tensor.matmul(h_ps[:, :], lhsT=aggT_sb[:, :],
                                     rhs=w1_sb[:, :], start=True, stop=False)
                    nc.tensor.matmul(h_ps[:, :], lhsT=ones_sb[:, 0:128],
                                     rhs=b1_sb[:, :], start=False, stop=True)
                    # leaky relu -> sbuf
                    # leaky(x) = NEG*x + (1-NEG)*relu(x); only one PSUM
                    # input allowed per DVE op, so relu goes via ACT first.
                    hr_sb = workp.tile([128, HID], f32, tag="hr_sb")
                    nc.scalar.activation(hr_sb[:, :], h_ps[:, :], AF.Relu,
                                         scale=1.0 - NEG)
                    h_sb = workp.tile([128, HID], mmdt, tag="h_sb")
                    nc.vector.scalar_tensor_tensor(
                        h_sb[:, :], in0=h_ps[:, :], scalar=NEG,
                        in1=hr_sb[:, :], op0=mybir.AluOpType.mult,
                        op1=mybir.AluOpType.add)
                    # pool accumulate
                    nc.tensor.matmul(
                        pooled_ps[:, :],
                        lhsT=pmat_sb[:, st * G:(st + 1) * G],
                        rhs=h_sb[:, :],
                        start=(st == 0), stop=(st == NST - 1),
                        skip_group_check=True)

            # ---------------- tail: allreduce + MLP
            pooled_sb = workp.tile([G, HID], f32, tag="pooled")
            nc.vector.tensor_copy(out=pooled_sb[:, :], in_=pooled_ps[:, :])

            plsum = workp.tile([G, HID], f32, tag="plsum")
            if cfg.get("NO_CC"):
                nc.vector.tensor_copy(out=plsum[:, :], in_=pooled_sb[:, :])
            else:
                cc_in = dramp.tile([G, HID], f32)
                cc_out = dramp.tile([G, HID], f32)
                nc.gpsimd.dma_start(out=cc_in[:, :], in_=pooled_sb[:, :])
                nc.gpsimd.collective_compute(
                    "AllReduce", mybir.AluOpType.add,
                    replica_groups=[list(range(NC))],
                    ins=[cc_in[:, :].opt()], outs=[cc_out[:, :].opt()])
                nc.gpsimd.dma_start(out=plsum[:, :], in_=cc_out[:, :])

            cnt_sb = workp.tile([G, 1], f32, tag="cnt")
            nc.sync.dma_start(out=cnt_sb[:, :], in_=cnt_d[:, :])
            rec_sb = workp.tile([G, 1], f32, tag="rec")
            nc.vector.reciprocal(rec_sb[:, :], cnt_sb[:, :])
            pm_sb = workp.tile([G, HID], mmdt, tag="pm")
            nc.vector.tensor_scalar_mul(pm_sb[:, :], plsum[:, :], rec_sb[:, :])

            # transpose pooled -> z^T rows [128, G] halves
            zt = []
            for jj in range(HID // 128):
                tp = ps_t.tile([128, G], mmdt, tag="aggT")
                nc.tensor.transpose(tp[:, :], pm_sb[:, jj * 128:(jj + 1) * 128],
                                    iden_sb[0:G, 0:G])
                t_sb = workp.tile([128, G], mmdt, tag=f"zt{jj}")
                nc.scalar.copy(out=t_sb[:, :], in_=tp[:, :])
                zt.append(t_sb)
            # class-embedding^T [EH, G]
            emb_sb = workp.tile([NCLS, EH], mmdt, tag="emb")
            nc.sync.dma_start(out=emb_sb[:, :], in_=emb_d[:, :])
            clt_sb = workp.tile([NCLS, G], mmdt, tag="clt")
            nc.sync.dma_start(out=clt_sb[:, :], in_=clt_d[:, :])
            ce_ps = ps_t.tile([EH, G], f32, tag="aggT")
            nc.tensor.matmul(ce_ps[:, :], lhsT=emb_sb[:, :], rhs=clt_sb[:, :],
                             start=True, stop=True)
            ce_sb = workp.tile([EH, G], mmdt, tag="ce_sb")
            nc.scalar.copy(out=ce_sb[:, :], in_=ce_ps[:, :])
            zt.append(ce_sb)

            # W2: z2^T[128j] = sum_k W2blk[k,j].T @ zt[k]
            w2_sb = workp.tile([128, 6 * 128], mmdt, tag="w2")
            nc.sync.dma_start(out=w2_sb[:, :], in_=w2_d[:, :])
            b2_sb = workp.tile([1, HID], mmdt, tag="b2")
            nc.sync.dma_start(out=b2_sb[:, :], in_=b2_d[:, :])
            ones_g = workp.tile([1, G], mmdt, tag="onesg")
            nc.vector.memset(ones_g[:, :], 1.0)
            nk = (HID + EH) // 128
            z2 = []
            for jj in range(2):
                zp = ps_h.tile([128, G], f32, tag="h")
                for kk in range(nk):
                    nc.tensor.matmul(
                        zp[:, :],
                        lhsT=w2_sb[:, (kk * 2 + jj) * 128:(kk * 2 + jj + 1) * 128],
                        rhs=zt[kk][:, :], start=(kk == 0), stop=False)
                nc.tensor.matmul(zp[:, :], lhsT=b2_sb[:, jj * 128:(jj + 1) * 128],
                                 rhs=ones_g[:, :], start=False, stop=True)
                zr_sb = workp.tile([128, G], f32, tag="zr_sb")
                nc.scalar.activation(zr_sb[:, :], zp[:, :], AF.Relu,
                                     scale=1.0 - NEG)
                z_sb = workp.tile([128, G], mmdt, tag=f"z2sb{jj}")
                nc.vector.scalar_tensor_tensor(
                    z_sb[:, :], in0=zp[:, :], scalar=NEG, in1=zr_sb[:, :],
                    op0=mybir.AluOpType.mult, op1=mybir.AluOpType.add)
                z2.append(z_sb)

            w3_sb = workp.tile([128, 2], mmdt, tag="w3")
            nc.sync.dma_start(out=w3_sb[:, :], in_=w3_d[:, :])
            b3_sb = workp.tile([1, 1], mmdt, tag="b3")
            nc.sync.dma_start(out=b3_sb[:, :], in_=b3_d[:, :])
            op = ps_h.tile([1, G], f32, tag="h")
            for jj in range(2):
                nc.tensor.matmul(op[:, :], lhsT=w3_sb[:, jj:jj + 1],
                                 rhs=z2[jj][:, :], start=(jj == 0), stop=False)
            nc.tensor.matmul(op[:, :], lhsT=b3_sb[:, :], rhs=ones_g[:, :],
                             start=False, stop=True)
            o_sb = workp.tile([1, G], f32, tag="osb")
            nc.vector.tensor_copy(out=o_sb[:, :], in_=op[:, :])
            nc.sync.dma_start(out=out_d[:, :], in_=o_sb[:, :])

    return nc


# ================================================================= runner
def _run(inputs, cfg=None, trace=False):
    from concourse.bass_utils import run_bass_kernel_spmd
    cfg = dict(CFG if cfg is None else cfg)
    prep = host_prep(inputs, cfg)
    nc = build(prep.static)
    nc.finalize()
    res = run_bass_kernel_spmd(
        nc, prep.in_maps, core_ids=list(range(cfg["NCORES"])), trace=trace)
    out = np.asarray(res.results[0]["out"], np.float32).reshape(-1, 1)
    return out, res


def kernel(**inputs):
    out, _ = _run(inputs)
    return out



# revision 7
# speedup vs baseline: 3.2628x; 3.2628x over previous
"""GCN discriminator kernel for Trainium2 (8 NeuronCores, SPMD).

Math (matching the reference):
  deg[n]  = sum_{e: dst=n} w_e + 1
  dinv    = 1/sqrt(deg)
  norm_e  = dinv[src]*w_e*dinv[dst];  self-loop n: dinv[n]^2
  agg     = sum over incoming edges of norm_e * x[src]         [N, 128]
  h       = leaky_relu(agg @ W1 + b1)                          [N, 256]
  pooled  = segment_mean(h, batch)                             [64, 256]
  z       = leaky_relu(concat(pooled, emb[cls]) @ W2 + b2)
  out     = z @ W3 + b3                                        [64, 1]

Sharding: `batch` is sorted, so nodes are range-partitioned at GRAPH
boundaries — core c owns graphs [8c, 8c+8) and their node range.  Each
core aggregates its own destinations, pools its own 8 graphs, runs the
tiny MLP on them, and writes out[1, 8].  The host concatenates the 8
per-core outputs; no collective at all.

Aggregation per 128-dst supertile: edge slots are gathered (HBM
dma_gather, 4 SWDGE queues round-robin so descriptor generation uses
all 8 Q7 cores) into [128 slot, F] chunks; the chunk is the stationary
matmul operand against a one-hot scatter matrix S [slot, dst], giving
aggT [F, dst] directly (no transpose pass).  Self loops are one linear
DMA per supertile (a diagonal S chunk), never gathered.
"""

import numpy as np
import ml_dtypes

# ----------------------------------------------------------------- config
CFG = dict(
    N=50000, F=128, HID=256, G=64, NCLS=10,
    NCORES=8,
    XLO=32768,            # rows in the "lo" x tensor (int16-indexable)
    K=128,                # edge slots per chunk (matmul contraction)
    NEG=0.2,
    NQ=4,                 # SWDGE queues to cycle gathers over
    WARM=3,               # leading supertiles that 0-pad (vs -1 strip)
    GDT="bf16",           # gather dtype: "f32" | "bf16"  (x rows + S weights)
    MMDT="bf16",          # downstream matmul dtype
    GRAN=1,               # kept for test.py printout (per-supertile gathers)
    NO_GATHER=0,          # debug: memset instead of dma_gather
)


def _np_dt(s):
    return {"f32": np.float32, "bf16": ml_dtypes.bfloat16}[s]


# ================================================================= host prep
class Prep:
    pass


def host_prep(inputs, cfg):
    """Integer/layout preprocessing + normalization weights."""
    N, F, G = cfg["N"], cfg["F"], cfg["G"]
    NC, XLO, K = cfg["NCORES"], cfg["XLO"], cfg["K"]
    WARM = cfg["WARM"]
    GPC = G // NC  # graphs per core

    x = np.asarray(inputs["x"], np.float32)
    ei = np.asarray(inputs["edge_index"], np.int64)
    ew = np.asarray(inputs["edge_weight"], np.float32)
    batch = np.asarray(inputs["batch"], np.int64)
    cls = np.asarray(inputs["class_labels"], np.int64)
    W1 = np.asarray(inputs["W1"], np.float32)
    b1 = np.asarray(inputs["b1"], np.float32)
    emb = np.asarray(inputs["emb"], np.float32)
    W2 = np.asarray(inputs["W2"], np.float32)
    b2 = np.asarray(inputs["b2"], np.float32)
    W3 = np.asarray(inputs["W3"], np.float32)
    b3 = np.asarray(inputs["b3"], np.float32)

    HID = W1.shape[1]
    EH = emb.shape[1]

    # --- normalization weights ------------------------------------------
    row, col = ei[0], ei[1]
    deg = np.zeros(N, np.float64)
    np.add.at(deg, col, ew.astype(np.float64))
    deg += 1.0
    dinv = 1.0 / np.sqrt(deg)
    wnorm = (dinv[row] * ew.astype(np.float64) * dinv[col]).astype(np.float32)
    loopw = (dinv * dinv).astype(np.float32)

    # --- graph-aligned node ranges --------------------------------------
    gcnt = np.bincount(batch, minlength=G)
    B = np.zeros(G + 1, np.int64)
    np.cumsum(gcnt, out=B[1:])
    cb = B[::GPC]                       # core boundaries, len NC+1
    Dc = np.diff(cb)
    NST = int(-(-Dc.max() // K))
    NDL = NST * K

    # --- bucket edges by (core, supertile, half) -------------------------
    dst_g = np.searchsorted(B, col, side="right") - 1
    core_of = dst_g // GPC
    dst_loc = col - cb[core_of]
    st_of = dst_loc // K
    j_of = (dst_loc % K).astype(np.int32)
    half_of = (row >= XLO).astype(np.int64)
    srcl = np.where(half_of == 1, row - XLO, row).astype(np.int32)

    key = (core_of * NST + st_of) * 2 + half_of
    order = np.argsort(key, kind="stable")
    srcl_s = srcl[order]
    j_s = j_of[order]
    w_s = wnorm[order]

    nbuckets = NC * NST * 2
    cnt = np.bincount(key[order], minlength=nbuckets).reshape(NC, NST, 2)
    starts = np.zeros(nbuckets + 1, np.int64)
    np.cumsum(cnt.reshape(-1), out=starts[1:])

    CH = -(-cnt // K)          # ceil chunks per (core, st, half)
    CH = CH.max(axis=0)        # [NST, 2] static max over cores
    NCHMAX = int(CH.sum(axis=1).max()) + 1     # +1 loop chunk
    nch_st = CH.sum(axis=1) + 1                # chunks per st (compact)
    TOTCH = int(nch_st.sum())

    # idx column layout: per st [lo cols][hi cols]; warm sts extend hi
    # (or lo when no hi region) to cover the full NCHMAX-1 gather area.
    clo_ch = CH[:, 0].astype(np.int64)              # lo chunks gathered
    chi_ch = CH[:, 1].astype(np.int64)              # hi chunks gathered
    chi_eff = chi_ch.copy()
    for st in range(min(WARM, NST)):
        chi_eff[st] = (NCHMAX - 1) - clo_ch[st]
    col_off = np.zeros(NST + 1, np.int64)
    np.cumsum((clo_ch + chi_eff) * (K // 16), out=col_off[1:])
    NCOLS = int(col_off[-1])
    smat_off = np.zeros(NST + 1, np.int64)
    np.cumsum(nch_st, out=smat_off[1:])

    static = dict(cfg=cfg, NST=NST, NDL=NDL, NCHMAX=NCHMAX, TOTCH=TOTCH,
                  NCOLS=NCOLS, HID=HID, EH=EH, GPC=GPC,
                  clo_ch=clo_ch, chi_ch=chi_ch, chi_eff=chi_eff,
                  col_off=col_off, smat_off=smat_off, nch_st=nch_st)

    # --- per-core tensors ------------------------------------------------
    gdt = _np_dt(cfg["GDT"])
    mmdt = _np_dt(cfg["MMDT"])

    xlo = np.ascontiguousarray(x[:XLO]).astype(gdt)
    xhi = np.ascontiguousarray(x[XLO:]).astype(gdt)
    w2blk = np.zeros((128, 6 * 128), np.float32)
    for kk in range(3):
        for jj in range(2):
            w2blk[:, (kk * 2 + jj) * 128:(kk * 2 + jj + 1) * 128] = \
                W2[kk * 128:(kk + 1) * 128, jj * 128:(jj + 1) * 128]
    w3m = np.zeros((128, 2), np.float32)
    w3m[:, 0] = W3[0:128, 0]
    w3m[:, 1] = W3[128:256, 0]

    in_maps = []
    for c in range(NC):
        n0, n1 = int(cb[c]), int(cb[c + 1])
        gidx = np.zeros((NCOLS * 16,), np.int16)
        gidx_view = gidx.reshape(NCOLS, 16)
        smat = np.zeros((128, TOTCH * K), gdt)
        pmat = np.zeros((128, NST * GPC), mmdt)
        xown = np.zeros((NDL, F), gdt)
        xown[:n1 - n0] = x[n0:n1].astype(gdt)

        for st in range(NST):
            warm = st < WARM
            # ---- gathered halves
            for half in (0, 1):
                b = (c * NST + st) * 2 + half
                s0, s1 = starts[b], starts[b + 1]
                seg_src = srcl_s[s0:s1]
                seg_j = j_s[s0:s1]
                seg_w = w_s[s0:s1]
                nch_half = int(clo_ch[st] if half == 0 else chi_eff[st])
                L = nch_half * K
                if L == 0:
                    continue
                idxs = np.full(L, 0, np.int16)  # safe pad: gathers row 0
                idxs[:len(seg_src)] = seg_src[:L]
                co = col_off[st] + (0 if half == 0 else clo_ch[st] * (K // 16))
                gidx_view[co:co + nch_half * (K // 16), :] = \
                    idxs.reshape(nch_half * (K // 16), 16)
                # S chunks (compact order: lo chunks, then real hi chunks)
                base_ci = smat_off[st] + (0 if half == 0 else clo_ch[st])
                nreal = int(clo_ch[st] if half == 0 else chi_ch[st])
                for k in range(nreal):
                    jj = seg_j[k * K:(k + 1) * K]
                    ww = seg_w[k * K:(k + 1) * K]
                    rows = np.arange(len(jj))
                    smat[rows, (base_ci + k) * K + jj] = ww.astype(gdt)
            # ---- loop (self-edge) diagonal chunk
            ci_loop = smat_off[st] + nch_st[st] - 1
            nodes = np.arange(n0 + st * K, min(n0 + (st + 1) * K, n1))
            p = nodes - (n0 + st * K)
            smat[p, ci_loop * K + p] = loopw[nodes].astype(gdt)
            # ---- pooling matrix
            pmat[p, st * GPC + (batch[nodes] - c * GPC)] = 1.0

        gidx2 = np.tile(gidx_view.T, (8, 1))   # [128, NCOLS] replicated

        counts = np.maximum(gcnt[c * GPC:(c + 1) * GPC], 1).astype(np.float32)
        ceT = np.ascontiguousarray(
            emb[cls[c * GPC:(c + 1) * GPC]].T).astype(mmdt)   # [EH, GPC]

        m = dict(
            xlo=xlo, xhi=xhi, xown=xown,
            gidx=np.ascontiguousarray(gidx2),
            smat=smat,
            pmat=pmat,
            w1=W1.astype(mmdt),
            w2blk=w2blk.astype(mmdt),
            w3=w3m.astype(mmdt),
            b1=b1.reshape(1, HID).astype(mmdt),
            b2=b2.reshape(1, HID).astype(mmdt),
            b3=b3.reshape(1, 1).astype(mmdt),
            ceT=ceT,
            counts=counts.reshape(GPC, 1),
        )
        in_maps.append(m)

    prep = Prep()
    prep.static = static
    prep.in_maps = in_maps
    return prep


# ================================================================= builder
def build(static):
    import concourse.bass as bass
    from concourse import bacc, tile
    import concourse.mybir as mybir

    cfg = static["cfg"]
    N, F = cfg["N"], cfg["F"]
    XLO, K = cfg["XLO"], cfg["K"]
    NEG = cfg["NEG"]
    NQ = cfg["NQ"]
    NST, NDL, NCHMAX = static["NST"], static["NDL"], static["NCHMAX"]
    TOTCH, NCOLS = static["TOTCH"], static["NCOLS"]
    HID, EH, GPC = static["HID"], static["EH"], static["GPC"]
    NCLS = cfg["NCLS"]
    clo_ch, chi_ch = static["clo_ch"], static["chi_ch"]
    chi_eff, col_off = static["chi_eff"], static["col_off"]
    smat_off, nch_st = static["smat_off"], static["nch_st"]

    gdt = {"f32": mybir.dt.float32, "bf16": mybir.dt.bfloat16}[cfg["GDT"]]
    mmdt = {"f32": mybir.dt.float32, "bf16": mybir.dt.bfloat16}[cfg["MMDT"]]
    f32 = mybir.dt.float32
    AF = mybir.ActivationFunctionType

    nc = bacc.Bacc(None, target_bir_lowering=False, debug=True,
                   num_swdge_queues=NQ)

    xlo_d = nc.declare_dram_parameter("xlo", [XLO, F], gdt, isOutput=False)
    xhi_d = nc.declare_dram_parameter("xhi", [N - XLO, F], gdt, isOutput=False)
    xown_d = nc.declare_dram_parameter("xown", [NDL, F], gdt, isOutput=False)
    gidx_d = nc.declare_dram_parameter("gidx", [128, NCOLS], mybir.dt.int16, isOutput=False)
    smat_d = nc.declare_dram_parameter("smat", [128, TOTCH * K], gdt, isOutput=False)
    pmat_d = nc.declare_dram_parameter("pmat", [128, NST * GPC], mmdt, isOutput=False)
    w1_d = nc.declare_dram_parameter("w1", [F, HID], mmdt, isOutput=False)
    w2_d = nc.declare_dram_parameter("w2blk", [128, 6 * 128], mmdt, isOutput=False)
    w3_d = nc.declare_dram_parameter("w3", [128, 2], mmdt, isOutput=False)
    b1_d = nc.declare_dram_parameter("b1", [1, HID], mmdt, isOutput=False)
    b2_d = nc.declare_dram_parameter("b2", [1, HID], mmdt, isOutput=False)
    b3_d = nc.declare_dram_parameter("b3", [1, 1], mmdt, isOutput=False)
    ceT_d = nc.declare_dram_parameter("ceT", [EH, GPC], mmdt, isOutput=False)
    cnt_d = nc.declare_dram_parameter("counts", [GPC, 1], f32, isOutput=False)
    out_d = nc.declare_dram_parameter("out", [1, GPC], f32, isOutput=True)

    iden_np = np.eye(128, dtype=_np_dt(cfg["MMDT"]))
    iden_d = nc.inline_tensor(iden_np, name="iden")

    with tile.TileContext(nc) as tc:
        with (
            tc.tile_pool(name="const", bufs=1) as constp,
            tc.tile_pool(name="gat", bufs=3) as gatp,
            tc.tile_pool(name="smp", bufs=3) as smp,
            tc.tile_pool(name="work", bufs=4) as workp,
            tc.tile_pool(name="ps_agg", bufs=2, space="PSUM") as ps_agg,
            tc.tile_pool(name="ps_h", bufs=2, space="PSUM") as ps_h,
            tc.tile_pool(name="ps_t", bufs=2, space="PSUM") as ps_t,
            tc.tile_pool(name="ps_pool", bufs=1, space="PSUM") as ps_pool,
        ):
            # ---- persistent SBUF loads
            gidx_sb = constp.tile([128, NCOLS], mybir.dt.int16)
            nc.sync.dma_start(out=gidx_sb[:, :], in_=gidx_d[:, :])
            pmat_sb = constp.tile([128, NST * GPC], mmdt)
            nc.sync.dma_start(out=pmat_sb[:, :], in_=pmat_d[:, :])
            w1_sb = constp.tile([F, HID], mmdt)
            nc.sync.dma_start(out=w1_sb[:, :], in_=w1_d[:, :])
            iden_sb = constp.tile([128, 128], mmdt)
            nc.sync.dma_start(out=iden_sb[:, :], in_=iden_d[:, :])
            b1_sb = constp.tile([1, HID], mmdt)
            nc.sync.dma_start(out=b1_sb[:, :], in_=b1_d[:, :])
            ones_sb = constp.tile([1, 128], mmdt)
            nc.vector.memset(ones_sb[:, :], 1.0)

            pooled_ps = ps_pool.tile([GPC, HID], f32)

            qload = [0] * NQ   # greedy least-loaded queue assignment
            for st in range(NST):
                nch = int(nch_st[st])
                clo = int(clo_ch[st])
                chie = int(chi_eff[st])
                gt = gatp.tile([128, NCHMAX, F], gdt, tag="gt")
                c0 = int(col_off[st])
                if cfg.get("NO_GATHER"):
                    nc.vector.memset(gt[:, :, :], 0.125)
                else:
                    if clo > 0:
                        q = min(range(NQ), key=lambda i: qload[i])
                        qload[q] += clo * K
                        nc.gpsimd.dma_gather(
                            gt[:, 0:clo, :], xlo_d[:, :],
                            gidx_sb[:, c0:c0 + clo * (K // 16)],
                            num_idxs=clo * K, num_idxs_reg=clo * K,
                            elem_size=F, single_packet=False,
                            queue_num=q)
                    if chie > 0:
                        q = min(range(NQ), key=lambda i: qload[i])
                        qload[q] += chie * K
                        nc.gpsimd.dma_gather(
                            gt[:, clo:clo + chie, :], xhi_d[:, :],
                            gidx_sb[:, c0 + clo * (K // 16):
                                    c0 + (clo + chie) * (K // 16)],
                            num_idxs=chie * K, num_idxs_reg=chie * K,
                            elem_size=F, single_packet=False,
                            queue_num=q)
                # self-loop (diagonal) chunk: linear load of own dst rows
                nc.sync.dma_start(out=gt[:, NCHMAX - 1, :],
                                  in_=xown_d[st * K:(st + 1) * K, :])

                sm = smp.tile([128, NCHMAX * K], gdt, tag="sm")
                m0 = int(smat_off[st])
                nc.sync.dma_start(
                    out=sm[:, 0:nch * K],
                    in_=smat_d[:, m0 * K:(m0 + nch) * K])

                # ---- aggregation: aggT[F, dst] = sum_ci gt_ci^T-free @ S_ci
                aggT = ps_agg.tile([128, 128], f32, tag="aggT")
                for ci in range(nch):
                    gt_ci = NCHMAX - 1 if ci == nch - 1 else \
                        (ci if ci < clo else clo + (ci - clo))
                    nc.tensor.matmul(
                        aggT[:, :],
                        lhsT=gt[:, gt_ci, :],
                        rhs=sm[:, ci * K:(ci + 1) * K],
                        start=(ci == 0), stop=(ci == nch - 1))
                aggT_sb = workp.tile([128, 128], mmdt, tag="aggT_sb")
                nc.scalar.copy(out=aggT_sb[:, :], in_=aggT[:, :])

                # ---- W1 + b1, leaky relu
                h_ps = ps_h.tile([128, HID], f32, tag="h")
                nc.tensor.matmul(h_ps[:, :], lhsT=aggT_sb[:, :],
                                 rhs=w1_sb[:, :], start=True, stop=False)
                nc.tensor.matmul(h_ps[:, :], lhsT=ones_sb[:, 0:128],
                                 rhs=b1_sb[:, :], start=False, stop=True)
                hr_sb = workp.tile([128, HID], f32, tag="hr_sb")
                nc.scalar.activation(hr_sb[:, :], h_ps[:, :], AF.Relu,
                                     scale=1.0 - NEG)
                h_sb = workp.tile([128, HID], mmdt, tag="h_sb")
                nc.vector.scalar_tensor_tensor(
                    h_sb[:, :], in0=h_ps[:, :], scalar=NEG,
                    in1=hr_sb[:, :], op0=mybir.AluOpType.mult,
                    op1=mybir.AluOpType.add)
                # ---- pool accumulate (graphs of this core only)
                nc.tensor.matmul(
                    pooled_ps[:, :],
                    lhsT=pmat_sb[:, st * GPC:(st + 1) * GPC],
                    rhs=h_sb[:, :],
                    start=(st == 0), stop=(st == NST - 1),
                    skip_group_check=True)

            # ---------------- tail: per-core MLP on its own GPC graphs
            pooled_sb = workp.tile([GPC, HID], f32, tag="pooled")
            nc.vector.tensor_copy(out=pooled_sb[:, :], in_=pooled_ps[:, :])
            cnt_sb = workp.tile([GPC, 1], f32, tag="cnt")
            nc.sync.dma_start(out=cnt_sb[:, :], in_=cnt_d[:, :])
            rec_sb = workp.tile([GPC, 1], f32, tag="rec")
            nc.vector.reciprocal(rec_sb[:, :], cnt_sb[:, :])
            pm_sb = workp.tile([GPC, HID], mmdt, tag="pm")
            nc.vector.tensor_scalar_mul(pm_sb[:, :], pooled_sb[:, :],
                                        rec_sb[:, :])

            # transpose pooled -> [128, GPC] halves
            zt = []
            for jj in range(HID // 128):
                tp = ps_t.tile([128, GPC], mmdt, tag="tp")
                nc.tensor.transpose(tp[:, :],
                                    pm_sb[:, jj * 128:(jj + 1) * 128],
                                    iden_sb[0:GPC, 0:GPC])
                t_sb = workp.tile([128, GPC], mmdt, tag=f"zt{jj}")
                nc.scalar.copy(out=t_sb[:, :], in_=tp[:, :])
                zt.append(t_sb)
            ceT_sb = workp.tile([EH, GPC], mmdt, tag="ceT")
            nc.sync.dma_start(out=ceT_sb[:, :], in_=ceT_d[:, :])
            zt.append(ceT_sb)

            # W2 (+b2), leaky
            w2_sb = workp.tile([128, 6 * 128], mmdt, tag="w2")
            nc.sync.dma_start(out=w2_sb[:, :], in_=w2_d[:, :])
            b2_sb = workp.tile([1, HID], mmdt, tag="b2")
            nc.sync.dma_start(out=b2_sb[:, :], in_=b2_d[:, :])
            ones_g = workp.tile([1, GPC], mmdt, tag="onesg")
            nc.vector.memset(ones_g[:, :], 1.0)
            nk = (HID + EH) // 128
            z2 = []
            for jj in range(2):
                zp = ps_h.tile([128, GPC], f32, tag="h")
                for kk in range(nk):
                    nc.tensor.matmul(
                        zp[:, :],
                        lhsT=w2_sb[:, (kk * 2 + jj) * 128:(kk * 2 + jj + 1) * 128],
                        rhs=zt[kk][:, :], start=(kk == 0), stop=False)
                nc.tensor.matmul(zp[:, :], lhsT=b2_sb[:, jj * 128:(jj + 1) * 128],
                                 rhs=ones_g[:, :], start=False, stop=True)
                zr_sb = workp.tile([128, GPC], f32, tag="zr_sb")
                nc.scalar.activation(zr_sb[:, :], zp[:, :], AF.Relu,
                                     scale=1.0 - NEG)
                z_sb = workp.tile([128, GPC], mmdt, tag=f"z2sb{jj}")
                nc.vector.scalar_tensor_tensor(
                    z_sb[:, :], in0=zp[:, :], scalar=NEG, in1=zr_sb[:, :],
                    op0=mybir.AluOpType.mult, op1=mybir.AluOpType.add)
                z2.append(z_sb)

            # W3 (+b3)
            w3_sb = workp.tile([128, 2], mmdt, tag="w3")
            nc.sync.dma_start(out=w3_sb[:, :], in_=w3_d[:, :])
            b3_sb = workp.tile([1, 1], mmdt, tag="b3")
            nc.sync.dma_start(out=b3_sb[:, :], in_=b3_d[:, :])
            op = ps_h.tile([1, GPC], f32, tag="h")
            for jj in range(2):
                nc.tensor.matmul(op[:, :], lhsT=w3_sb[:, jj:jj + 1],
                                 rhs=z2[jj][:, :], start=(jj == 0), stop=False)
            nc.tensor.matmul(op[:, :], lhsT=b3_sb[:, :], rhs=ones_g[:, :],
                             start=False, stop=True)
            o_sb = workp.tile([1, GPC], f32, tag="osb")
            nc.vector.tensor_copy(out=o_sb[:, :], in_=op[:, :])
            nc.sync.dma_start(out=out_d[:, :], in_=o_sb[:, :])

    return nc


# ================================================================= runner
def _run(inputs, cfg=None, trace=False):
    from concourse.bass_utils import run_bass_kernel_spmd
    cfg = dict(CFG if cfg is None else cfg)
    for k in ("NQ", "WARM", "GRAN", "NO_GATHER", "N", "F", "HID", "G",
              "NCLS", "NCORES", "XLO", "K"):
        cfg[k] = int(cfg[k])
    prep = host_prep(inputs, cfg)
    nc = build(prep.static)
    nc.finalize()
    res = run_bass_kernel_spmd(
        nc, prep.in_maps, core_ids=list(range(cfg["NCORES"])), trace=trace)
    parts = [np.asarray(res.results[c]["out"], np.float32).reshape(-1)
             for c in range(cfg["NCORES"])]
    out = np.concatenate(parts).reshape(-1, 1)
    return out, res


def kernel(**inputs):
    out, _ = _run(inputs)
    return out


# revision 21
# speedup vs baseline: 4.0265x; 1.2341x over previous
"""GCN discriminator kernel for Trainium2 (8 NeuronCores, SPMD).

Math (matching the reference):
  deg[n]  = sum_{e: dst=n} w_e + 1
  dinv    = 1/sqrt(deg)
  norm_e  = dinv[src]*w_e*dinv[dst];  self-loop n: dinv[n]^2
  agg     = sum over incoming edges of norm_e * x[src]         [N, 128]
  h       = leaky_relu(agg @ W1 + b1)                          [N, 256]
  pooled  = segment_mean(h, batch)                             [64, 256]
  z       = leaky_relu(concat(pooled, emb[cls]) @ W2 + b2)
  out     = z @ W3 + b3                                        [64, 1]

Sharding: `batch` is sorted, so nodes are range-partitioned at GRAPH
boundaries — core c owns graphs [8c, 8c+8) and their node range.  Each
core aggregates its own destinations, pools its own 8 graphs, runs the
tiny MLP on them, and writes out[1, 8].  The host concatenates the 8
per-core outputs; no collective at all.

Aggregation per 128-dst supertile: edge slots are gathered (HBM
dma_gather, 4 SWDGE queues round-robin so descriptor generation uses
all 8 Q7 cores) into [128 slot, F] chunks; the chunk is the stationary
matmul operand against a one-hot scatter matrix S [slot, dst], giving
aggT [F, dst] directly (no transpose pass).  Self loops are one linear
DMA per supertile (a diagonal S chunk), never gathered.
"""

import numpy as np
import ml_dtypes

# ----------------------------------------------------------------- config
CFG = dict(
    N=50000, F=128, HID=256, G=64, NCLS=10,
    NCORES=8,
    XLO=32768,            # rows in the "lo" x tensor (int16-indexable)
    K=128,                # edge slots per chunk (matmul contraction)
    NEG=0.2,
    NQ=4,                 # SWDGE queues to cycle gathers over
    WARM=6,               # leading supertiles that 0-pad (vs -1 strip);
                          # must equal the gt pool depth (buffer warm-up)
    BUFS=6,               # gt/sm tile pool depth
    GDT="bf16",           # gather dtype: "f32" | "bf16"  (x rows + S weights)
    MMDT="bf16",          # downstream matmul dtype
    GRAN=1,               # kept for test.py printout (per-supertile gathers)
    NO_GATHER=0,          # debug: memset instead of dma_gather
)


def _np_dt(s):
    return {"f32": np.float32, "bf16": ml_dtypes.bfloat16}[s]


# ================================================================= host prep
class Prep:
    pass


def host_prep(inputs, cfg):
    """Integer/layout preprocessing + normalization weights."""
    N, F, G = cfg["N"], cfg["F"], cfg["G"]
    NC, XLO, K = cfg["NCORES"], cfg["XLO"], cfg["K"]
    WARM = cfg["WARM"]
    GPC = G // NC  # graphs per core

    x = np.asarray(inputs["x"], np.float32)
    ei = np.asarray(inputs["edge_index"], np.int64)
    ew = np.asarray(inputs["edge_weight"], np.float32)
    batch = np.asarray(inputs["batch"], np.int64)
    cls = np.asarray(inputs["class_labels"], np.int64)
    W1 = np.asarray(inputs["W1"], np.float32)
    b1 = np.asarray(inputs["b1"], np.float32)
    emb = np.asarray(inputs["emb"], np.float32)
    W2 = np.asarray(inputs["W2"], np.float32)
    b2 = np.asarray(inputs["b2"], np.float32)
    W3 = np.asarray(inputs["W3"], np.float32)
    b3 = np.asarray(inputs["b3"], np.float32)

    HID = W1.shape[1]
    EH = emb.shape[1]

    # --- normalization weights ------------------------------------------
    row, col = ei[0], ei[1]
    deg = np.zeros(N, np.float64)
    np.add.at(deg, col, ew.astype(np.float64))
    deg += 1.0
    dinv = 1.0 / np.sqrt(deg)
    wnorm = (dinv[row] * ew.astype(np.float64) * dinv[col]).astype(np.float32)
    loopw = (dinv * dinv).astype(np.float32)

    # --- graph-aligned node ranges --------------------------------------
    gcnt = np.bincount(batch, minlength=G)
    B = np.zeros(G + 1, np.int64)
    np.cumsum(gcnt, out=B[1:])
    cb = B[::GPC]                       # core boundaries, len NC+1
    Dc = np.diff(cb)
    NST = int(-(-Dc.max() // K))
    NDL = NST * K

    # --- bucket edges by (core, supertile, half) -------------------------
    dst_g = np.searchsorted(B, col, side="right") - 1
    core_of = dst_g // GPC
    dst_loc = col - cb[core_of]
    st_of = dst_loc // K
    j_of = (dst_loc % K).astype(np.int32)
    half_of = (row >= XLO).astype(np.int64)
    srcl = np.where(half_of == 1, row - XLO, row).astype(np.int32)

    key = (core_of * NST + st_of) * 2 + half_of
    order = np.argsort(key, kind="stable")
    srcl_s = srcl[order]
    j_s = j_of[order]
    w_s = wnorm[order]

    nbuckets = NC * NST * 2
    cnt = np.bincount(key[order], minlength=nbuckets).reshape(NC, NST, 2)
    starts = np.zeros(nbuckets + 1, np.int64)
    np.cumsum(cnt.reshape(-1), out=starts[1:])

    CH = -(-cnt // K)          # ceil chunks per (core, st, half)
    CH = CH.max(axis=0)        # [NST, 2] static max over cores
    NCHMAX = int(CH.sum(axis=1).max()) + 1     # +1 loop chunk
    nch_st = CH.sum(axis=1) + 1                # chunks per st (compact)
    TOTCH = int(nch_st.sum())

    # idx column layout: per st [lo cols][hi cols]; warm sts extend hi
    # (or lo when no hi region) to cover the full NCHMAX-1 gather area.
    clo_ch = CH[:, 0].astype(np.int64)              # lo chunks gathered
    chi_ch = CH[:, 1].astype(np.int64)              # hi chunks gathered
    chi_eff = chi_ch.copy()
    for st in range(min(WARM, NST)):
        chi_eff[st] = (NCHMAX - 1) - clo_ch[st]
    col_off = np.zeros(NST + 1, np.int64)
    np.cumsum((clo_ch + chi_eff) * (K // 16), out=col_off[1:])
    NCOLS = int(col_off[-1])
    smat_off = np.zeros(NST + 1, np.int64)
    np.cumsum(nch_st, out=smat_off[1:])

    # static gather-call list (st, half) in emission order
    calls = []
    for st in range(NST):
        if clo_ch[st] > 0:
            calls.append((st, 0))
        if chi_eff[st] > 0:
            calls.append((st, 1))
    call_of = {c: i for i, c in enumerate(calls)}
    NCALLS = len(calls)

    static = dict(cfg=cfg, NST=NST, NDL=NDL, NCHMAX=NCHMAX, TOTCH=TOTCH,
                  NCOLS=NCOLS, HID=HID, EH=EH, GPC=GPC,
                  clo_ch=clo_ch, chi_ch=chi_ch, chi_eff=chi_eff,
                  col_off=col_off, smat_off=smat_off, nch_st=nch_st,
                  calls=calls, call_of=call_of, NCALLS=NCALLS)

    # --- per-core tensors ------------------------------------------------
    gdt = _np_dt(cfg["GDT"])
    mmdt = _np_dt(cfg["MMDT"])

    xlo = np.ascontiguousarray(x[:XLO]).astype(gdt)
    xhi = np.ascontiguousarray(x[XLO:]).astype(gdt)
    w2blk = np.zeros((128, 6 * 128), np.float32)
    for kk in range(3):
        for jj in range(2):
            w2blk[:, (kk * 2 + jj) * 128:(kk * 2 + jj + 1) * 128] = \
                W2[kk * 128:(kk + 1) * 128, jj * 128:(jj + 1) * 128]
    w3m = np.zeros((128, 2), np.float32)
    w3m[:, 0] = W3[0:128, 0]
    w3m[:, 1] = W3[128:256, 0]

    in_maps = []
    for c in range(NC):
        n0, n1 = int(cb[c]), int(cb[c + 1])
        gidx = np.zeros((NCOLS * 16,), np.int16)
        gidx_view = gidx.reshape(NCOLS, 16)
        smat = np.zeros((128, TOTCH * K), gdt)
        pmat = np.zeros((128, NST * GPC), mmdt)
        xown = np.zeros((NDL, F), gdt)
        xown[:n1 - n0] = x[n0:n1].astype(gdt)
        ncnt = np.zeros((1, NCALLS), np.int32)

        for st in range(NST):
            warm = st < WARM
            # ---- gathered halves
            for half in (0, 1):
                b = (c * NST + st) * 2 + half
                s0, s1 = starts[b], starts[b + 1]
                seg_src = srcl_s[s0:s1]
                seg_j = j_s[s0:s1]
                seg_w = w_s[s0:s1]
                nch_half = int(clo_ch[st] if half == 0 else chi_eff[st])
                L = nch_half * K
                if L == 0:
                    continue
                # warm sts pad with 0 (fully writes the tile buffer, so
                # later -1-stripped gathers only leave finite stale data);
                # the rest pad with -1 which the Q7 ucode strips for free,
                # with num_idxs_reg carrying the true per-core count.
                idxs = np.full(L, (0 if warm else -1), np.int16)
                idxs[:len(seg_src)] = seg_src[:L]
                ncnt[0, call_of[(st, half)]] = L if warm else min(len(seg_src), L)
                co = col_off[st] + (0 if half == 0 else clo_ch[st] * (K // 16))
                gidx_view[co:co + nch_half * (K // 16), :] = \
                    idxs.reshape(nch_half * (K // 16), 16)
                # S chunks (compact order: lo chunks, then real hi chunks)
                base_ci = smat_off[st] + (0 if half == 0 else clo_ch[st])
                nreal = int(clo_ch[st] if half == 0 else chi_ch[st])
                for k in range(nreal):
                    jj = seg_j[k * K:(k + 1) * K]
                    ww = seg_w[k * K:(k + 1) * K]
                    rows = np.arange(len(jj))
                    smat[rows, (base_ci + k) * K + jj] = ww.astype(gdt)
            # ---- loop (self-edge) diagonal chunk
            ci_loop = smat_off[st] + nch_st[st] - 1
            nodes = np.arange(n0 + st * K, min(n0 + (st + 1) * K, n1))
            p = nodes - (n0 + st * K)
            smat[p, ci_loop * K + p] = loopw[nodes].astype(gdt)
            # ---- pooling matrix
            pmat[p, st * GPC + (batch[nodes] - c * GPC)] = 1.0

        gidx2 = np.tile(gidx_view.T, (8, 1))   # [128, NCOLS] replicated

        counts = np.maximum(gcnt[c * GPC:(c + 1) * GPC], 1).astype(np.float32)
        ceT = np.ascontiguousarray(
            emb[cls[c * GPC:(c + 1) * GPC]].T).astype(mmdt)   # [EH, GPC]

        m = dict(
            xlo=xlo, xhi=xhi, xown=xown, ncnt=ncnt,
            gidx=np.ascontiguousarray(gidx2),
            smat=smat,
            pmat=pmat,
            w1=W1.astype(mmdt),
            w2blk=w2blk.astype(mmdt),
            w3=w3m.astype(mmdt),
            b1=b1.reshape(1, HID).astype(mmdt),
            b2=b2.reshape(1, HID).astype(mmdt),
            b3=b3.reshape(1, 1).astype(mmdt),
            ceT=ceT,
            counts=counts.reshape(GPC, 1),
        )
        in_maps.append(m)

    prep = Prep()
    prep.static = static
    prep.in_maps = in_maps
    return prep


# ================================================================= builder
def build(static):
    import concourse.bass as bass
    from concourse import bacc, tile
    import concourse.mybir as mybir

    cfg = static["cfg"]
    N, F = cfg["N"], cfg["F"]
    XLO, K = cfg["XLO"], cfg["K"]
    NEG = cfg["NEG"]
    NQ = cfg["NQ"]
    NST, NDL, NCHMAX = static["NST"], static["NDL"], static["NCHMAX"]
    TOTCH, NCOLS = static["TOTCH"], static["NCOLS"]
    HID, EH, GPC = static["HID"], static["EH"], static["GPC"]
    NCLS = cfg["NCLS"]
    clo_ch, chi_ch = static["clo_ch"], static["chi_ch"]
    chi_eff, col_off = static["chi_eff"], static["col_off"]
    smat_off, nch_st = static["smat_off"], static["nch_st"]
    call_of, NCALLS = static["call_of"], static["NCALLS"]
    WARM, BUFS = cfg["WARM"], cfg["BUFS"]

    gdt = {"f32": mybir.dt.float32, "bf16": mybir.dt.bfloat16}[cfg["GDT"]]
    mmdt = {"f32": mybir.dt.float32, "bf16": mybir.dt.bfloat16}[cfg["MMDT"]]
    f32 = mybir.dt.float32
    AF = mybir.ActivationFunctionType

    nc = bacc.Bacc(None, target_bir_lowering=False, debug=True,
                   num_swdge_queues=NQ)

    xlo_d = nc.declare_dram_parameter("xlo", [XLO, F], gdt, isOutput=False)
    xhi_d = nc.declare_dram_parameter("xhi", [N - XLO, F], gdt, isOutput=False)
    xown_d = nc.declare_dram_parameter("xown", [NDL, F], gdt, isOutput=False)
    gidx_d = nc.declare_dram_parameter("gidx", [128, NCOLS], mybir.dt.int16, isOutput=False)
    smat_d = nc.declare_dram_parameter("smat", [128, TOTCH * K], gdt, isOutput=False)
    pmat_d = nc.declare_dram_parameter("pmat", [128, NST * GPC], mmdt, isOutput=False)
    w1_d = nc.declare_dram_parameter("w1", [F, HID], mmdt, isOutput=False)
    w2_d = nc.declare_dram_parameter("w2blk", [128, 6 * 128], mmdt, isOutput=False)
    w3_d = nc.declare_dram_parameter("w3", [128, 2], mmdt, isOutput=False)
    b1_d = nc.declare_dram_parameter("b1", [1, HID], mmdt, isOutput=False)
    b2_d = nc.declare_dram_parameter("b2", [1, HID], mmdt, isOutput=False)
    b3_d = nc.declare_dram_parameter("b3", [1, 1], mmdt, isOutput=False)
    ceT_d = nc.declare_dram_parameter("ceT", [EH, GPC], mmdt, isOutput=False)
    cnt_d = nc.declare_dram_parameter("counts", [GPC, 1], f32, isOutput=False)
    ncnt_d = nc.declare_dram_parameter("ncnt", [1, NCALLS], mybir.dt.int32, isOutput=False)
    out_d = nc.declare_dram_parameter("out", [1, GPC], f32, isOutput=True)

    iden_np = np.eye(128, dtype=_np_dt(cfg["MMDT"]))
    iden_d = nc.inline_tensor(iden_np, name="iden")

    with tile.TileContext(nc) as tc:
        with (
            tc.tile_pool(name="const", bufs=1) as constp,
            tc.tile_pool(name="gat", bufs=BUFS) as gatp,
            tc.tile_pool(name="smp", bufs=BUFS) as smp,
            tc.tile_pool(name="work", bufs=4) as workp,
            tc.tile_pool(name="ps_agg", bufs=2, space="PSUM") as ps_agg,
            tc.tile_pool(name="ps_h", bufs=2, space="PSUM") as ps_h,
            tc.tile_pool(name="ps_t", bufs=2, space="PSUM") as ps_t,
            tc.tile_pool(name="ps_pool", bufs=1, space="PSUM") as ps_pool,
        ):
            # ---- persistent SBUF loads (gidx split so the first gathers
            # don't wait on the whole table)
            gidx_sb = constp.tile([128, NCOLS], mybir.dt.int16)
            gsplit = [int(col_off[min(s, NST)])
                      for s in (0, 4, 12, 28, NST)]
            for a, b in zip(gsplit, gsplit[1:]):
                if b > a:
                    nc.sync.dma_start(out=gidx_sb[:, a:b], in_=gidx_d[:, a:b])
            ncnt_sb = constp.tile([1, NCALLS], mybir.dt.int32)
            nc.sync.dma_start(out=ncnt_sb[:, :], in_=ncnt_d[:, :])
            pmat_sb = constp.tile([128, NST * GPC], mmdt)
            nc.sync.dma_start(out=pmat_sb[:, :], in_=pmat_d[:, :])
            w1_sb = constp.tile([F, HID], mmdt)
            nc.sync.dma_start(out=w1_sb[:, :], in_=w1_d[:, :])
            iden_sb = constp.tile([128, 128], mmdt)
            nc.sync.dma_start(out=iden_sb[:, :], in_=iden_d[:, :])
            b1_sb = constp.tile([1, HID], mmdt)
            nc.sync.dma_start(out=b1_sb[:, :], in_=b1_d[:, :])
            ones_sb = constp.tile([1, 128], mmdt)
            nc.vector.memset(ones_sb[:, :], 1.0)
            # tail constants, loaded up front so the tail never waits on DMA
            cnt_sb = constp.tile([GPC, 1], f32)
            nc.sync.dma_start(out=cnt_sb[:, :], in_=cnt_d[:, :])
            ceT_sb = constp.tile([EH, GPC], mmdt)
            nc.sync.dma_start(out=ceT_sb[:, :], in_=ceT_d[:, :])
            w2_sb = constp.tile([128, 6 * 128], mmdt)
            nc.sync.dma_start(out=w2_sb[:, :], in_=w2_d[:, :])
            b2_sb = constp.tile([1, HID], mmdt)
            nc.sync.dma_start(out=b2_sb[:, :], in_=b2_d[:, :])
            w3_sb = constp.tile([128, 2], mmdt)
            nc.sync.dma_start(out=w3_sb[:, :], in_=w3_d[:, :])
            b3_sb = constp.tile([1, 1], mmdt)
            nc.sync.dma_start(out=b3_sb[:, :], in_=b3_d[:, :])

            pooled_ps = ps_pool.tile([GPC, HID], f32)

            # registers for per-core true gather lengths
            cnt_regs = [nc.gpsimd.alloc_register(f"gcnt{i}") for i in range(4)]
            nreg = 0

            qload = [0] * NQ   # greedy least-loaded queue assignment
            for st in range(NST):
                nch = int(nch_st[st])
                clo = int(clo_ch[st])
                chie = int(chi_eff[st])
                gt = gatp.tile([128, NCHMAX, F], gdt, tag="gt")
                c0 = int(col_off[st])
                if cfg.get("NO_GATHER"):
                    nc.vector.memset(gt[:, :, :], 0.125)
                else:
                    if clo > 0:
                        q = min(range(NQ), key=lambda i: qload[i])
                        qload[q] += clo * K
                        ci_call = call_of[(st, 0)]
                        reg = cnt_regs[nreg % len(cnt_regs)]
                        nreg += 1
                        nc.gpsimd.reg_load(reg, ncnt_sb[0:1, ci_call:ci_call + 1])
                        nc.gpsimd.dma_gather(
                            gt[:, 0:clo, :], xlo_d[:, :],
                            gidx_sb[:, c0:c0 + clo * (K // 16)],
                            num_idxs=clo * K, num_idxs_reg=reg,
                            elem_size=F, single_packet=False,
                            queue_num=q)
                    if chie > 0:
                        q = min(range(NQ), key=lambda i: qload[i])
                        qload[q] += chie * K
                        ci_call = call_of[(st, 1)]
                        reg = cnt_regs[nreg % len(cnt_regs)]
                        nreg += 1
                        nc.gpsimd.reg_load(reg, ncnt_sb[0:1, ci_call:ci_call + 1])
                        nc.gpsimd.dma_gather(
                            gt[:, clo:clo + chie, :], xhi_d[:, :],
                            gidx_sb[:, c0 + clo * (K // 16):
                                    c0 + (clo + chie) * (K // 16)],
                            num_idxs=chie * K, num_idxs_reg=reg,
                            elem_size=F, single_packet=False,
                            queue_num=q)
                # self-loop (diagonal) chunk: linear load of own dst rows
                nc.sync.dma_start(out=gt[:, NCHMAX - 1, :],
                                  in_=xown_d[st * K:(st + 1) * K, :])

                sm = smp.tile([128, NCHMAX * K], gdt, tag="sm")
                m0 = int(smat_off[st])
                nc.sync.dma_start(
                    out=sm[:, 0:nch * K],
                    in_=smat_d[:, m0 * K:(m0 + nch) * K])

                # ---- aggregation: aggT[F, dst] = sum_ci gt_ci^T-free @ S_ci
                aggT = ps_agg.tile([128, 128], f32, tag="aggT")
                for ci in range(nch):
                    gt_ci = NCHMAX - 1 if ci == nch - 1 else \
                        (ci if ci < clo else clo + (ci - clo))
                    nc.tensor.matmul(
                        aggT[:, :],
                        lhsT=gt[:, gt_ci, :],
                        rhs=sm[:, ci * K:(ci + 1) * K],
                        start=(ci == 0), stop=(ci == nch - 1))
                aggT_sb = workp.tile([128, 128], mmdt, tag="aggT_sb")
                nc.scalar.copy(out=aggT_sb[:, :], in_=aggT[:, :])

                # ---- W1 + b1, leaky relu
                h_ps = ps_h.tile([128, HID], f32, tag="h")
                nc.tensor.matmul(h_ps[:, :], lhsT=aggT_sb[:, :],
                                 rhs=w1_sb[:, :], start=True, stop=False)
                nc.tensor.matmul(h_ps[:, :], lhsT=ones_sb[:, 0:128],
                                 rhs=b1_sb[:, :], start=False, stop=True)
                hr_sb = workp.tile([128, HID], f32, tag="hr_sb")
                nc.scalar.activation(hr_sb[:, :], h_ps[:, :], AF.Relu,
                                     scale=1.0 - NEG)
                h_sb = workp.tile([128, HID], mmdt, tag="h_sb")
                nc.vector.scalar_tensor_tensor(
                    h_sb[:, :], in0=h_ps[:, :], scalar=NEG,
                    in1=hr_sb[:, :], op0=mybir.AluOpType.mult,
                    op1=mybir.AluOpType.add)
                # ---- pool accumulate (graphs of this core only)
                nc.tensor.matmul(
                    pooled_ps[:, :],
                    lhsT=pmat_sb[:, st * GPC:(st + 1) * GPC],
                    rhs=h_sb[:, :],
                    start=(st == 0), stop=(st == NST - 1),
                    skip_group_check=True)

            # ---------------- tail: per-core MLP on its own GPC graphs
            pooled_sb = workp.tile([GPC, HID], f32, tag="pooled")
            nc.vector.tensor_copy(out=pooled_sb[:, :], in_=pooled_ps[:, :])
            rec_sb = workp.tile([GPC, 1], f32, tag="rec")
            nc.vector.reciprocal(rec_sb[:, :], cnt_sb[:, :])
            pm_sb = workp.tile([GPC, HID], mmdt, tag="pm")
            nc.vector.tensor_scalar_mul(pm_sb[:, :], pooled_sb[:, :],
                                        rec_sb[:, :])

            # transpose pooled -> [128, GPC] halves
            zt = []
            for jj in range(HID // 128):
                tp = ps_t.tile([128, GPC], mmdt, tag="tp")
                nc.tensor.transpose(tp[:, :],
                                    pm_sb[:, jj * 128:(jj + 1) * 128],
                                    iden_sb[0:GPC, 0:GPC])
                t_sb = workp.tile([128, GPC], mmdt, tag=f"zt{jj}")
                nc.scalar.copy(out=t_sb[:, :], in_=tp[:, :])
                zt.append(t_sb)
            zt.append(ceT_sb)

            # W2 (+b2), leaky
            ones_g = workp.tile([1, GPC], mmdt, tag="onesg")
            nc.vector.memset(ones_g[:, :], 1.0)
            nk = (HID + EH) // 128
            z2 = []
            for jj in range(2):
                zp = ps_h.tile([128, GPC], f32, tag="h")
                for kk in range(nk):
                    nc.tensor.matmul(
                        zp[:, :],
                        lhsT=w2_sb[:, (kk * 2 + jj) * 128:(kk * 2 + jj + 1) * 128],
                        rhs=zt[kk][:, :], start=(kk == 0), stop=False)
                nc.tensor.matmul(zp[:, :], lhsT=b2_sb[:, jj * 128:(jj + 1) * 128],
                                 rhs=ones_g[:, :], start=False, stop=True)
                zr_sb = workp.tile([128, GPC], f32, tag="zr_sb")
                nc.scalar.activation(zr_sb[:, :], zp[:, :], AF.Relu,
                                     scale=1.0 - NEG)
                z_sb = workp.tile([128, GPC], mmdt, tag=f"z2sb{jj}")
                nc.vector.scalar_tensor_tensor(
                    z_sb[:, :], in0=zp[:, :], scalar=NEG, in1=zr_sb[:, :],
                    op0=mybir.AluOpType.mult, op1=mybir.AluOpType.add)
                z2.append(z_sb)

            # W3 (+b3)
            op = ps_h.tile([1, GPC], f32, tag="h")
            for jj in range(2):
                nc.tensor.matmul(op[:, :], lhsT=w3_sb[:, jj:jj + 1],
                                 rhs=z2[jj][:, :], start=(jj == 0), stop=False)
            nc.tensor.matmul(op[:, :], lhsT=b3_sb[:, :], rhs=ones_g[:, :],
                             start=False, stop=True)
            o_sb = workp.tile([1, GPC], f32, tag="osb")
            nc.vector.tensor_copy(out=o_sb[:, :], in_=op[:, :])
            nc.sync.dma_start(out=out_d[:, :], in_=o_sb[:, :])

    return nc


# ================================================================= runner
def _run(inputs, cfg=None, trace=False):
    from concourse.bass_utils import run_bass_kernel_spmd
    cfg = dict(CFG if cfg is None else cfg)
    for k in ("NQ", "WARM", "GRAN", "NO_GATHER", "N", "F", "HID", "G",
              "NCLS", "NCORES", "XLO", "K", "BUFS"):
        cfg[k] = int(cfg[k])
    assert cfg["WARM"] >= cfg["BUFS"], "warm-up must cover the pool depth"
    prep = host_prep(inputs, cfg)
    nc = build(prep.static)
    nc.finalize()
    res = run_bass_kernel_spmd(
        nc, prep.in_maps, core_ids=list(range(cfg["NCORES"])), trace=trace)
    parts = [np.asarray(res.results[c]["out"], np.float32).reshape(-1)
             for c in range(cfg["NCORES"])]
    out = np.concatenate(parts).reshape(-1, 1)
    return out, res


def kernel(**inputs):
    out, _ = _run(inputs)
    return out


# revision 33
# speedup vs baseline: 4.7326x; 1.1754x over previous
"""GCN discriminator kernel for Trainium2 (8 NeuronCores, SPMD).

Math (matching the reference):
  deg[n]  = sum_{e: dst=n} w_e + 1
  dinv    = 1/sqrt(deg)
  norm_e  = dinv[src]*w_e*dinv[dst];  self-loop n: dinv[n]^2
  agg     = sum over incoming edges of norm_e * x[src]         [N, 128]
  h       = leaky_relu(agg @ W1 + b1)                          [N, 256]
  pooled  = segment_mean(h, batch)                             [64, 256]
  z       = leaky_relu(concat(pooled, emb[cls]) @ W2 + b2)
  out     = z @ W3 + b3                                        [64, 1]

Sharding: `batch` is sorted, so nodes are range-partitioned at GRAPH
boundaries — core c owns graphs [8c, 8c+8) and their node range.  Each
core aggregates its own destinations, pools its own 8 graphs, runs the
tiny MLP on them, and writes out[1, 8].  The host concatenates the 8
per-core outputs; no collective at all.

Aggregation per 128-dst supertile: edge slots are gathered (HBM
dma_gather, 4 SWDGE queues round-robin so descriptor generation uses
all 8 Q7 cores) into [128 slot, F] chunks; the chunk is the stationary
matmul operand against a one-hot scatter matrix S [slot, dst], giving
aggT [F, dst] directly (no transpose pass).  Self loops are one linear
DMA per supertile (a diagonal S chunk), never gathered.
"""

import numpy as np
import ml_dtypes

# ----------------------------------------------------------------- config
CFG = dict(
    N=50000, F=128, HID=256, G=64, NCLS=10,
    NCORES=8,
    XLO=32768,            # rows in the "lo" x tensor (int16-indexable)
    K=128,                # edge slots per chunk (matmul contraction)
    NEG=0.2,
    NQ=4,                 # SWDGE queues to cycle gathers over
    WARM=0,               # unused (tiles are memset on first use)
    BUFS=8,               # gt/sm tile pool depth
    GDT="bf16",           # gather dtype: "f32" | "bf16"  (x rows + S weights)
    MMDT="bf16",          # downstream matmul dtype
    GRAN=1,               # kept for test.py printout (per-supertile gathers)
    NO_GATHER=0,          # debug: memset instead of dma_gather
)


def _np_dt(s):
    return {"f32": np.float32, "bf16": ml_dtypes.bfloat16}[s]


# ================================================================= host prep
class Prep:
    pass


def host_prep(inputs, cfg):
    """Integer/layout preprocessing + normalization weights."""
    N, F, G = cfg["N"], cfg["F"], cfg["G"]
    NC, XLO, K = cfg["NCORES"], cfg["XLO"], cfg["K"]
    WARM = cfg["WARM"]
    GPC = G // NC  # graphs per core

    x = np.asarray(inputs["x"], np.float32)
    ei = np.asarray(inputs["edge_index"], np.int64)
    ew = np.asarray(inputs["edge_weight"], np.float32)
    batch = np.asarray(inputs["batch"], np.int64)
    cls = np.asarray(inputs["class_labels"], np.int64)
    W1 = np.asarray(inputs["W1"], np.float32)
    b1 = np.asarray(inputs["b1"], np.float32)
    emb = np.asarray(inputs["emb"], np.float32)
    W2 = np.asarray(inputs["W2"], np.float32)
    b2 = np.asarray(inputs["b2"], np.float32)
    W3 = np.asarray(inputs["W3"], np.float32)
    b3 = np.asarray(inputs["b3"], np.float32)

    HID = W1.shape[1]
    EH = emb.shape[1]

    # --- normalization weights ------------------------------------------
    row, col = ei[0], ei[1]
    deg = np.zeros(N, np.float64)
    np.add.at(deg, col, ew.astype(np.float64))
    deg += 1.0
    dinv = 1.0 / np.sqrt(deg)
    wnorm = (dinv[row] * ew.astype(np.float64) * dinv[col]).astype(np.float32)
    loopw = (dinv * dinv).astype(np.float32)

    # --- graph-aligned node ranges --------------------------------------
    gcnt = np.bincount(batch, minlength=G)
    B = np.zeros(G + 1, np.int64)
    np.cumsum(gcnt, out=B[1:])
    cb = B[::GPC]                       # core boundaries, len NC+1
    Dc = np.diff(cb)
    NST = int(-(-Dc.max() // K))
    NDL = NST * K

    # --- bucket edges by (core, supertile, half) -------------------------
    dst_g = np.searchsorted(B, col, side="right") - 1
    core_of = dst_g // GPC
    dst_loc = col - cb[core_of]
    st_of = dst_loc // K
    j_of = (dst_loc % K).astype(np.int32)
    half_of = (row >= XLO).astype(np.int64)
    srcl = np.where(half_of == 1, row - XLO, row).astype(np.int32)

    key = (core_of * NST + st_of) * 2 + half_of
    order = np.argsort(key, kind="stable")
    srcl_s = srcl[order]
    j_s = j_of[order]
    w_s = wnorm[order]

    nbuckets = NC * NST * 2
    cnt = np.bincount(key[order], minlength=nbuckets).reshape(NC, NST, 2)
    starts = np.zeros(nbuckets + 1, np.int64)
    np.cumsum(cnt.reshape(-1), out=starts[1:])

    CH = -(-cnt // K)          # ceil chunks per (core, st, half)
    CH = CH.max(axis=0)        # [NST, 2] static max over cores
    NCHMAX = int(CH.sum(axis=1).max()) + 1     # +1 loop chunk
    nch_st = CH.sum(axis=1) + 1                # chunks per st (compact)
    TOTCH = int(nch_st.sum())

    # idx column layout: per st [lo cols][hi cols]
    clo_ch = CH[:, 0].astype(np.int64)              # lo chunks gathered
    chi_ch = CH[:, 1].astype(np.int64)              # hi chunks gathered
    chi_eff = chi_ch
    col_off = np.zeros(NST + 1, np.int64)
    np.cumsum((clo_ch + chi_eff) * (K // 16), out=col_off[1:])
    NCOLS = int(col_off[-1])
    smat_off = np.zeros(NST + 1, np.int64)
    np.cumsum(nch_st, out=smat_off[1:])

    # supertile emission order: largest first (better queue packing; the
    # final gather + its DMA drain are the smallest), and the static
    # gather-call list (st, half) in that order
    st_order = sorted(range(NST),
                      key=lambda s: -(int(clo_ch[s]) + int(chi_ch[s])))
    calls = []
    for st in st_order:
        if clo_ch[st] > 0:
            calls.append((st, 0))
        if chi_eff[st] > 0:
            calls.append((st, 1))
    call_of = {c: i for i, c in enumerate(calls)}
    NCALLS = len(calls)

    static = dict(cfg=cfg, NST=NST, NDL=NDL, NCHMAX=NCHMAX, TOTCH=TOTCH,
                  NCOLS=NCOLS, HID=HID, EH=EH, GPC=GPC,
                  clo_ch=clo_ch, chi_ch=chi_ch, chi_eff=chi_eff,
                  col_off=col_off, smat_off=smat_off, nch_st=nch_st,
                  st_order=st_order, calls=calls, call_of=call_of,
                  NCALLS=NCALLS)

    # --- per-core tensors ------------------------------------------------
    gdt = _np_dt(cfg["GDT"])
    mmdt = _np_dt(cfg["MMDT"])

    xlo = np.ascontiguousarray(x[:XLO]).astype(gdt)
    xhi = np.ascontiguousarray(x[XLO:]).astype(gdt)
    w2blk = np.zeros((128, 6 * 128), np.float32)
    for kk in range(3):
        for jj in range(2):
            w2blk[:, (kk * 2 + jj) * 128:(kk * 2 + jj + 1) * 128] = \
                W2[kk * 128:(kk + 1) * 128, jj * 128:(jj + 1) * 128]
    w3m = np.zeros((128, 2), np.float32)
    w3m[:, 0] = W3[0:128, 0]
    w3m[:, 1] = W3[128:256, 0]

    in_maps = []
    for c in range(NC):
        n0, n1 = int(cb[c]), int(cb[c + 1])
        gidx = np.zeros((NCOLS * 16,), np.int16)
        gidx_view = gidx.reshape(NCOLS, 16)
        smat = np.zeros((128, TOTCH * K), gdt)
        pmat = np.zeros((128, NST * GPC), mmdt)
        xown = np.zeros((NDL, F), gdt)
        xown[:n1 - n0] = x[n0:n1].astype(gdt)
        ncnt = np.zeros((1, NCALLS), np.int32)

        for st in range(NST):
            # ---- gathered halves
            for half in (0, 1):
                b = (c * NST + st) * 2 + half
                s0, s1 = starts[b], starts[b + 1]
                seg_src = srcl_s[s0:s1]
                seg_j = j_s[s0:s1]
                seg_w = w_s[s0:s1]
                nch_half = int(clo_ch[st] if half == 0 else chi_eff[st])
                L = nch_half * K
                if L == 0:
                    continue
                # pad with -1: the Q7 ucode strips trailing negatives for
                # free; num_idxs_reg carries the true per-core count.
                # (gt tiles are memset once at first use, so the stripped
                # region only ever holds finite stale data.)
                idxs = np.full(L, -1, np.int16)
                idxs[:len(seg_src)] = seg_src[:L]
                ncnt[0, call_of[(st, half)]] = min(len(seg_src), L)
                co = col_off[st] + (0 if half == 0 else clo_ch[st] * (K // 16))
                gidx_view[co:co + nch_half * (K // 16), :] = \
                    idxs.reshape(nch_half * (K // 16), 16)
                # S chunks (compact order: lo chunks, then real hi chunks)
                base_ci = smat_off[st] + (0 if half == 0 else clo_ch[st])
                nreal = int(clo_ch[st] if half == 0 else chi_ch[st])
                for k in range(nreal):
                    jj = seg_j[k * K:(k + 1) * K]
                    ww = seg_w[k * K:(k + 1) * K]
                    rows = np.arange(len(jj))
                    smat[rows, (base_ci + k) * K + jj] = ww.astype(gdt)
            # ---- loop (self-edge) diagonal chunk
            ci_loop = smat_off[st] + nch_st[st] - 1
            nodes = np.arange(n0 + st * K, min(n0 + (st + 1) * K, n1))
            p = nodes - (n0 + st * K)
            smat[p, ci_loop * K + p] = loopw[nodes].astype(gdt)
            # ---- pooling matrix
            pmat[p, st * GPC + (batch[nodes] - c * GPC)] = 1.0

        gidx2 = np.tile(gidx_view.T, (8, 1))   # [128, NCOLS] replicated

        counts = np.maximum(gcnt[c * GPC:(c + 1) * GPC], 1).astype(np.float32)
        ceT = np.ascontiguousarray(
            emb[cls[c * GPC:(c + 1) * GPC]].T).astype(mmdt)   # [EH, GPC]

        m = dict(
            xlo=xlo, xhi=xhi, xown=xown, ncnt=ncnt,
            gidx=np.ascontiguousarray(gidx2),
            smat=smat,
            pmat=pmat,
            w1=W1.astype(mmdt),
            w2blk=w2blk.astype(mmdt),
            w3=w3m.astype(mmdt),
            b1=b1.reshape(1, HID).astype(mmdt),
            b2=b2.reshape(1, HID).astype(mmdt),
            b3=b3.reshape(1, 1).astype(mmdt),
            ceT=ceT,
            counts=counts.reshape(GPC, 1),
        )
        in_maps.append(m)

    prep = Prep()
    prep.static = static
    prep.in_maps = in_maps
    return prep


# ================================================================= builder
def build(static):
    import concourse.bass as bass
    from concourse import bacc, tile
    import concourse.mybir as mybir

    cfg = static["cfg"]
    N, F = cfg["N"], cfg["F"]
    XLO, K = cfg["XLO"], cfg["K"]
    NEG = cfg["NEG"]
    NQ = cfg["NQ"]
    NST, NDL, NCHMAX = static["NST"], static["NDL"], static["NCHMAX"]
    TOTCH, NCOLS = static["TOTCH"], static["NCOLS"]
    HID, EH, GPC = static["HID"], static["EH"], static["GPC"]
    NCLS = cfg["NCLS"]
    clo_ch, chi_ch = static["clo_ch"], static["chi_ch"]
    chi_eff, col_off = static["chi_eff"], static["col_off"]
    smat_off, nch_st = static["smat_off"], static["nch_st"]
    call_of, NCALLS = static["call_of"], static["NCALLS"]
    st_order = static["st_order"]
    BUFS = cfg["BUFS"]

    gdt = {"f32": mybir.dt.float32, "bf16": mybir.dt.bfloat16}[cfg["GDT"]]
    mmdt = {"f32": mybir.dt.float32, "bf16": mybir.dt.bfloat16}[cfg["MMDT"]]
    f32 = mybir.dt.float32
    AF = mybir.ActivationFunctionType

    nc = bacc.Bacc(None, target_bir_lowering=False, debug=True,
                   num_swdge_queues=NQ)

    xlo_d = nc.declare_dram_parameter("xlo", [XLO, F], gdt, isOutput=False)
    xhi_d = nc.declare_dram_parameter("xhi", [N - XLO, F], gdt, isOutput=False)
    xown_d = nc.declare_dram_parameter("xown", [NDL, F], gdt, isOutput=False)
    gidx_d = nc.declare_dram_parameter("gidx", [128, NCOLS], mybir.dt.int16, isOutput=False)
    smat_d = nc.declare_dram_parameter("smat", [128, TOTCH * K], gdt, isOutput=False)
    pmat_d = nc.declare_dram_parameter("pmat", [128, NST * GPC], mmdt, isOutput=False)
    w1_d = nc.declare_dram_parameter("w1", [F, HID], mmdt, isOutput=False)
    w2_d = nc.declare_dram_parameter("w2blk", [128, 6 * 128], mmdt, isOutput=False)
    w3_d = nc.declare_dram_parameter("w3", [128, 2], mmdt, isOutput=False)
    b1_d = nc.declare_dram_parameter("b1", [1, HID], mmdt, isOutput=False)
    b2_d = nc.declare_dram_parameter("b2", [1, HID], mmdt, isOutput=False)
    b3_d = nc.declare_dram_parameter("b3", [1, 1], mmdt, isOutput=False)
    ceT_d = nc.declare_dram_parameter("ceT", [EH, GPC], mmdt, isOutput=False)
    cnt_d = nc.declare_dram_parameter("counts", [GPC, 1], f32, isOutput=False)
    ncnt_d = nc.declare_dram_parameter("ncnt", [1, NCALLS], mybir.dt.int32, isOutput=False)
    out_d = nc.declare_dram_parameter("out", [1, GPC], f32, isOutput=True)

    iden_np = np.eye(128, dtype=_np_dt(cfg["MMDT"]))
    iden_d = nc.inline_tensor(iden_np, name="iden")

    with tile.TileContext(nc) as tc:
        with (
            tc.tile_pool(name="const", bufs=1) as constp,
            tc.tile_pool(name="gat", bufs=BUFS) as gatp,
            tc.tile_pool(name="smp", bufs=BUFS) as smp,
            tc.tile_pool(name="work", bufs=4) as workp,
            tc.tile_pool(name="ps_agg", bufs=2, space="PSUM") as ps_agg,
            tc.tile_pool(name="ps_h", bufs=2, space="PSUM") as ps_h,
            tc.tile_pool(name="ps_t", bufs=2, space="PSUM") as ps_t,
            tc.tile_pool(name="ps_pool", bufs=1, space="PSUM") as ps_pool,
        ):
            # ---- persistent SBUF loads.  gidx is split and spread over
            # engines so the first gathers don't wait on the whole table;
            # tail-only constants go to the idle Vector/Scalar queues.
            gidx_sb = constp.tile([128, NCOLS], mybir.dt.int16)
            first_sts = st_order[:4]
            for s in first_sts:
                a, b = int(col_off[s]), int(col_off[s + 1])
                if b > a:
                    nc.sync.dma_start(out=gidx_sb[:, a:b], in_=gidx_d[:, a:b])
            rest = sorted(set(range(NST)) - set(first_sts))
            runs = []
            for s in rest:
                a, b = int(col_off[s]), int(col_off[s + 1])
                if runs and runs[-1][1] == a:
                    runs[-1][1] = b
                else:
                    runs.append([a, b])
            for a, b in runs:
                if b > a:
                    nc.scalar.dma_start(out=gidx_sb[:, a:b], in_=gidx_d[:, a:b])
            ncnt_sb = constp.tile([1, NCALLS], mybir.dt.int32)
            nc.sync.dma_start(out=ncnt_sb[:, :], in_=ncnt_d[:, :])
            w1_sb = constp.tile([F, HID], mmdt)
            nc.sync.dma_start(out=w1_sb[:, :], in_=w1_d[:, :])
            b1_sb = constp.tile([1, HID], mmdt)
            nc.sync.dma_start(out=b1_sb[:, :], in_=b1_d[:, :])
            pmat_sb = constp.tile([128, NST * GPC], mmdt)
            nc.scalar.dma_start(out=pmat_sb[:, :], in_=pmat_d[:, :])
            iden_sb = constp.tile([128, 128], mmdt)
            nc.scalar.dma_start(out=iden_sb[:, :], in_=iden_d[:, :])
            ones_sb = constp.tile([1, 128], mmdt)
            nc.vector.memset(ones_sb[:, :], 1.0)
            # tail constants on the scalar queue (sync stays free for the
            # per-supertile loop/sm loads)
            cnt_sb = constp.tile([GPC, 1], f32)
            nc.scalar.dma_start(out=cnt_sb[:, :], in_=cnt_d[:, :])
            ceT_sb = constp.tile([EH, GPC], mmdt)
            nc.scalar.dma_start(out=ceT_sb[:, :], in_=ceT_d[:, :])
            w2_sb = constp.tile([128, 6 * 128], mmdt)
            nc.scalar.dma_start(out=w2_sb[:, :], in_=w2_d[:, :])
            b2_sb = constp.tile([1, HID], mmdt)
            nc.scalar.dma_start(out=b2_sb[:, :], in_=b2_d[:, :])
            w3_sb = constp.tile([128, 2], mmdt)
            nc.scalar.dma_start(out=w3_sb[:, :], in_=w3_d[:, :])
            b3_sb = constp.tile([1, 1], mmdt)
            nc.scalar.dma_start(out=b3_sb[:, :], in_=b3_d[:, :])

            pooled_ps = ps_pool.tile([GPC, HID], f32)

            # registers for per-core true gather lengths; ncnt is stored in
            # emission order, so consecutive calls batch-load RB at a time
            RB = 8
            cnt_regs = [nc.gpsimd.alloc_register(f"gcnt{i}") for i in range(RB)]
            call_i = 0

            def next_cnt_reg():
                nonlocal call_i
                if call_i % RB == 0:
                    lo = call_i
                    hi = min(call_i + RB, NCALLS)
                    nc.gpsimd.reg_load(cnt_regs[:hi - lo],
                                       ncnt_sb[0:1, lo:hi])
                reg = cnt_regs[call_i % RB]
                call_i += 1
                return reg

            qload = [0] * NQ   # greedy least-loaded queue assignment
            for sti, st in enumerate(st_order):
                nch = int(nch_st[st])
                clo = int(clo_ch[st])
                chie = int(chi_eff[st])
                gt = gatp.tile([128, NCHMAX, F], gdt, tag="gt")
                if sti < BUFS:
                    # first use of this pool buffer: zero it so -1-stripped
                    # gather tails only ever read finite stale data
                    nc.vector.memset(gt[:, :, :], 0.0)
                c0 = int(col_off[st])
                if cfg.get("NO_GATHER"):
                    nc.vector.memset(gt[:, :, :], 0.125)
                else:
                    if clo > 0:
                        q = min(range(NQ), key=lambda i: qload[i])
                        qload[q] += clo * K
                        nc.gpsimd.dma_gather(
                            gt[:, 0:clo, :], xlo_d[:, :],
                            gidx_sb[:, c0:c0 + clo * (K // 16)],
                            num_idxs=clo * K, num_idxs_reg=next_cnt_reg(),
                            elem_size=F, single_packet=False,
                            queue_num=q)
                    if chie > 0:
                        q = min(range(NQ), key=lambda i: qload[i])
                        qload[q] += chie * K
                        nc.gpsimd.dma_gather(
                            gt[:, clo:clo + chie, :], xhi_d[:, :],
                            gidx_sb[:, c0 + clo * (K // 16):
                                    c0 + (clo + chie) * (K // 16)],
                            num_idxs=chie * K, num_idxs_reg=next_cnt_reg(),
                            elem_size=F, single_packet=False,
                            queue_num=q)
                # self-loop (diagonal) chunk: linear load of own dst rows
                nc.sync.dma_start(out=gt[:, NCHMAX - 1, :],
                                  in_=xown_d[st * K:(st + 1) * K, :])

                sm = smp.tile([128, NCHMAX * K], gdt, tag="sm")
                m0 = int(smat_off[st])
                nc.sync.dma_start(
                    out=sm[:, 0:nch * K],
                    in_=smat_d[:, m0 * K:(m0 + nch) * K])

                # ---- aggregation: aggT[F, dst] = sum_ci gt_ci^T-free @ S_ci
                aggT = ps_agg.tile([128, 128], f32, tag="aggT")
                for ci in range(nch):
                    gt_ci = NCHMAX - 1 if ci == nch - 1 else \
                        (ci if ci < clo else clo + (ci - clo))
                    nc.tensor.matmul(
                        aggT[:, :],
                        lhsT=gt[:, gt_ci, :],
                        rhs=sm[:, ci * K:(ci + 1) * K],
                        start=(ci == 0), stop=(ci == nch - 1))
                aggT_sb = workp.tile([128, 128], mmdt, tag="aggT_sb")
                nc.scalar.copy(out=aggT_sb[:, :], in_=aggT[:, :])

                # ---- W1 + b1, leaky relu
                h_ps = ps_h.tile([128, HID], f32, tag="h")
                nc.tensor.matmul(h_ps[:, :], lhsT=aggT_sb[:, :],
                                 rhs=w1_sb[:, :], start=True, stop=False)
                nc.tensor.matmul(h_ps[:, :], lhsT=ones_sb[:, 0:128],
                                 rhs=b1_sb[:, :], start=False, stop=True)
                hr_sb = workp.tile([128, HID], f32, tag="hr_sb")
                nc.scalar.activation(hr_sb[:, :], h_ps[:, :], AF.Relu,
                                     scale=1.0 - NEG)
                h_sb = workp.tile([128, HID], mmdt, tag="h_sb")
                nc.vector.scalar_tensor_tensor(
                    h_sb[:, :], in0=h_ps[:, :], scalar=NEG,
                    in1=hr_sb[:, :], op0=mybir.AluOpType.mult,
                    op1=mybir.AluOpType.add)
                # ---- pool accumulate (graphs of this core only)
                nc.tensor.matmul(
                    pooled_ps[:, :],
                    lhsT=pmat_sb[:, st * GPC:(st + 1) * GPC],
                    rhs=h_sb[:, :],
                    start=(sti == 0), stop=(sti == NST - 1),
                    skip_group_check=True)

            # ---------------- tail: per-core MLP on its own GPC graphs
            pooled_sb = workp.tile([GPC, HID], f32, tag="pooled")
            nc.vector.tensor_copy(out=pooled_sb[:, :], in_=pooled_ps[:, :])
            rec_sb = workp.tile([GPC, 1], f32, tag="rec")
            nc.vector.reciprocal(rec_sb[:, :], cnt_sb[:, :])
            pm_sb = workp.tile([GPC, HID], mmdt, tag="pm")
            nc.vector.tensor_scalar_mul(pm_sb[:, :], pooled_sb[:, :],
                                        rec_sb[:, :])

            # transpose pooled -> [128, GPC] halves
            zt = []
            for jj in range(HID // 128):
                tp = ps_t.tile([128, GPC], mmdt, tag="tp")
                nc.tensor.transpose(tp[:, :],
                                    pm_sb[:, jj * 128:(jj + 1) * 128],
                                    iden_sb[0:GPC, 0:GPC])
                t_sb = workp.tile([128, GPC], mmdt, tag=f"zt{jj}")
                nc.scalar.copy(out=t_sb[:, :], in_=tp[:, :])
                zt.append(t_sb)
            zt.append(ceT_sb)

            # W2 (+b2), leaky
            ones_g = workp.tile([1, GPC], mmdt, tag="onesg")
            nc.vector.memset(ones_g[:, :], 1.0)
            nk = (HID + EH) // 128
            z2 = []
            for jj in range(2):
                zp = ps_h.tile([128, GPC], f32, tag="h")
                for kk in range(nk):
                    nc.tensor.matmul(
                        zp[:, :],
                        lhsT=w2_sb[:, (kk * 2 + jj) * 128:(kk * 2 + jj + 1) * 128],
                        rhs=zt[kk][:, :], start=(kk == 0), stop=False)
                nc.tensor.matmul(zp[:, :], lhsT=b2_sb[:, jj * 128:(jj + 1) * 128],
                                 rhs=ones_g[:, :], start=False, stop=True)
                zr_sb = workp.tile([128, GPC], f32, tag="zr_sb")
                nc.scalar.activation(zr_sb[:, :], zp[:, :], AF.Relu,
                                     scale=1.0 - NEG)
                z_sb = workp.tile([128, GPC], mmdt, tag=f"z2sb{jj}")
                nc.vector.scalar_tensor_tensor(
                    z_sb[:, :], in0=zp[:, :], scalar=NEG, in1=zr_sb[:, :],
                    op0=mybir.AluOpType.mult, op1=mybir.AluOpType.add)
                z2.append(z_sb)

            # W3 (+b3)
            op = ps_h.tile([1, GPC], f32, tag="h")
            for jj in range(2):
                nc.tensor.matmul(op[:, :], lhsT=w3_sb[:, jj:jj + 1],
                                 rhs=z2[jj][:, :], start=(jj == 0), stop=False)
            nc.tensor.matmul(op[:, :], lhsT=b3_sb[:, :], rhs=ones_g[:, :],
                             start=False, stop=True)
            o_sb = workp.tile([1, GPC], f32, tag="osb")
            nc.vector.tensor_copy(out=o_sb[:, :], in_=op[:, :])
            nc.sync.dma_start(out=out_d[:, :], in_=o_sb[:, :])

    return nc


# ================================================================= runner
def _run(inputs, cfg=None, trace=False):
    from concourse.bass_utils import run_bass_kernel_spmd
    cfg = dict(CFG if cfg is None else cfg)
    for k in ("NQ", "WARM", "GRAN", "NO_GATHER", "N", "F", "HID", "G",
              "NCLS", "NCORES", "XLO", "K", "BUFS"):
        cfg[k] = int(cfg[k])
    prep = host_prep(inputs, cfg)
    nc = build(prep.static)
    nc.finalize()
    res = run_bass_kernel_spmd(
        nc, prep.in_maps, core_ids=list(range(cfg["NCORES"])), trace=trace)
    parts = [np.asarray(res.results[c]["out"], np.float32).reshape(-1)
             for c in range(cfg["NCORES"])]
    out = np.concatenate(parts).reshape(-1, 1)
    return out, res


def kernel(**inputs):
    out, _ = _run(inputs)
    return out


# revision 39
# speedup vs baseline: 5.0535x; 1.0678x over previous
"""GCN discriminator kernel for Trainium2 (8 NeuronCores, SPMD).

Math (matching the reference):
  deg[n]  = sum_{e: dst=n} w_e + 1
  dinv    = 1/sqrt(deg)
  norm_e  = dinv[src]*w_e*dinv[dst];  self-loop n: dinv[n]^2
  agg     = sum over incoming edges of norm_e * x[src]         [N, 128]
  h       = leaky_relu(agg @ W1 + b1)                          [N, 256]
  pooled  = segment_mean(h, batch)                             [64, 256]
  z       = leaky_relu(concat(pooled, emb[cls]) @ W2 + b2)
  out     = z @ W3 + b3                                        [64, 1]

Sharding: `batch` is sorted, so nodes are range-partitioned at GRAPH
boundaries — core c owns graphs [8c, 8c+8) and their node range.  Each
core aggregates its own destinations, pools its own 8 graphs, runs the
tiny MLP on them, and writes out[1, 8].  The host concatenates the 8
per-core outputs; no collective at all.

Aggregation per 128-dst supertile: edge slots are gathered (HBM
dma_gather, 4 SWDGE queues round-robin so descriptor generation uses
all 8 Q7 cores) into [128 slot, F] chunks; the chunk is the stationary
matmul operand against a one-hot scatter matrix S [slot, dst], giving
aggT [F, dst] directly (no transpose pass).  Self loops are one linear
DMA per supertile (a diagonal S chunk), never gathered.
"""

import numpy as np
import ml_dtypes

# ----------------------------------------------------------------- config
CFG = dict(
    N=50000, F=128, HID=256, G=64, NCLS=10,
    NCORES=8,
    XLO=32768,            # rows in the "lo" x tensor (int16-indexable)
    K=128,                # edge slots per chunk (matmul contraction)
    NEG=0.2,
    NQ=4,                 # SWDGE queues to cycle gathers over
    WARM=0,               # unused (tiles are memset on first use)
    BUFS=8,               # gt/sm tile pool depth
    GDT="bf16",           # gather dtype: "f32" | "bf16"  (x rows + S weights)
    MMDT="bf16",          # downstream matmul dtype
    GRAN=1,               # kept for test.py printout (per-supertile gathers)
    NO_GATHER=0,          # debug: memset instead of dma_gather
)


def _np_dt(s):
    return {"f32": np.float32, "bf16": ml_dtypes.bfloat16}[s]


# ================================================================= host prep
class Prep:
    pass


def host_prep(inputs, cfg):
    """Integer/layout preprocessing + normalization weights."""
    N, F, G = cfg["N"], cfg["F"], cfg["G"]
    NC, XLO, K = cfg["NCORES"], cfg["XLO"], cfg["K"]
    WARM = cfg["WARM"]
    GPC = G // NC  # graphs per core

    x = np.asarray(inputs["x"], np.float32)
    ei = np.asarray(inputs["edge_index"], np.int64)
    ew = np.asarray(inputs["edge_weight"], np.float32)
    batch = np.asarray(inputs["batch"], np.int64)
    cls = np.asarray(inputs["class_labels"], np.int64)
    W1 = np.asarray(inputs["W1"], np.float32)
    b1 = np.asarray(inputs["b1"], np.float32)
    emb = np.asarray(inputs["emb"], np.float32)
    W2 = np.asarray(inputs["W2"], np.float32)
    b2 = np.asarray(inputs["b2"], np.float32)
    W3 = np.asarray(inputs["W3"], np.float32)
    b3 = np.asarray(inputs["b3"], np.float32)

    HID = W1.shape[1]
    EH = emb.shape[1]

    # --- normalization weights ------------------------------------------
    row, col = ei[0], ei[1]
    deg = np.zeros(N, np.float64)
    np.add.at(deg, col, ew.astype(np.float64))
    deg += 1.0
    dinv = 1.0 / np.sqrt(deg)
    wnorm = (dinv[row] * ew.astype(np.float64) * dinv[col]).astype(np.float32)
    loopw = (dinv * dinv).astype(np.float32)

    # --- graph-aligned node ranges --------------------------------------
    gcnt = np.bincount(batch, minlength=G)
    B = np.zeros(G + 1, np.int64)
    np.cumsum(gcnt, out=B[1:])
    cb = B[::GPC]                       # core boundaries, len NC+1
    Dc = np.diff(cb)
    NST = int(-(-Dc.max() // K))
    NDL = NST * K

    # --- bucket edges by (core, supertile, half) -------------------------
    dst_g = np.searchsorted(B, col, side="right") - 1
    core_of = dst_g // GPC
    dst_loc = col - cb[core_of]
    st_of = dst_loc // K
    j_of = (dst_loc % K).astype(np.int32)
    half_of = (row >= XLO).astype(np.int64)
    srcl = np.where(half_of == 1, row - XLO, row).astype(np.int32)

    key = (core_of * NST + st_of) * 2 + half_of
    order = np.argsort(key, kind="stable")
    srcl_s = srcl[order]
    j_s = j_of[order]
    w_s = wnorm[order]

    nbuckets = NC * NST * 2
    cnt = np.bincount(key[order], minlength=nbuckets).reshape(NC, NST, 2)
    starts = np.zeros(nbuckets + 1, np.int64)
    np.cumsum(cnt.reshape(-1), out=starts[1:])

    CH = -(-cnt // K)          # ceil chunks per (core, st, half)
    CH = CH.max(axis=0)        # [NST, 2] static max over cores
    NCHMAX = int(CH.sum(axis=1).max()) + 1     # +1 loop chunk
    nch_st = CH.sum(axis=1) + 1                # chunks per st (compact)
    TOTCH = int(nch_st.sum())

    # idx column layout: per st [lo cols][hi cols]
    clo_ch = CH[:, 0].astype(np.int64)              # lo chunks gathered
    chi_ch = CH[:, 1].astype(np.int64)              # hi chunks gathered
    chi_eff = chi_ch
    col_off = np.zeros(NST + 1, np.int64)
    np.cumsum((clo_ch + chi_eff) * (K // 16), out=col_off[1:])
    NCOLS = int(col_off[-1])
    smat_off = np.zeros(NST + 1, np.int64)
    np.cumsum(nch_st, out=smat_off[1:])

    # supertile emission order: largest first (better queue packing; the
    # final gather + its DMA drain are the smallest), and the static
    # gather-call list (st, half) in that order
    st_order = sorted(range(NST),
                      key=lambda s: -(int(clo_ch[s]) + int(chi_ch[s])))
    calls = []
    for st in st_order:
        if clo_ch[st] > 0:
            calls.append((st, 0))
        if chi_eff[st] > 0:
            calls.append((st, 1))
    call_of = {c: i for i, c in enumerate(calls)}
    NCALLS = len(calls)
    st_rank = {st: i for i, st in enumerate(st_order)}
    BUFS = cfg["BUFS"]

    static = dict(cfg=cfg, NST=NST, NDL=NDL, NCHMAX=NCHMAX, TOTCH=TOTCH,
                  NCOLS=NCOLS, HID=HID, EH=EH, GPC=GPC,
                  clo_ch=clo_ch, chi_ch=chi_ch, chi_eff=chi_eff,
                  col_off=col_off, smat_off=smat_off, nch_st=nch_st,
                  st_order=st_order, calls=calls, call_of=call_of,
                  NCALLS=NCALLS)

    # --- per-core tensors ------------------------------------------------
    gdt = _np_dt(cfg["GDT"])
    mmdt = _np_dt(cfg["MMDT"])

    xlo = np.ascontiguousarray(x[:XLO]).astype(gdt)
    xhi = np.ascontiguousarray(x[XLO:]).astype(gdt)
    w2blk = np.zeros((128, 6 * 128), np.float32)
    for kk in range(3):
        for jj in range(2):
            w2blk[:, (kk * 2 + jj) * 128:(kk * 2 + jj + 1) * 128] = \
                W2[kk * 128:(kk + 1) * 128, jj * 128:(jj + 1) * 128]
    w3m = np.zeros((128, 2), np.float32)
    w3m[:, 0] = W3[0:128, 0]
    w3m[:, 1] = W3[128:256, 0]

    in_maps = []
    for c in range(NC):
        n0, n1 = int(cb[c]), int(cb[c + 1])
        gidx = np.zeros((NCOLS * 16,), np.int16)
        gidx_view = gidx.reshape(NCOLS, 16)
        smat = np.zeros((128, TOTCH * K), gdt)
        pmat = np.zeros((128, NST * GPC), mmdt)
        xown = np.zeros((NDL, F), gdt)
        xown[:n1 - n0] = x[n0:n1].astype(gdt)
        ncnt = np.zeros((1, NCALLS), np.int32)

        for st in range(NST):
            # ---- gathered halves
            for half in (0, 1):
                b = (c * NST + st) * 2 + half
                s0, s1 = starts[b], starts[b + 1]
                seg_src = srcl_s[s0:s1]
                seg_j = j_s[s0:s1]
                seg_w = w_s[s0:s1]
                nch_half = int(clo_ch[st] if half == 0 else chi_eff[st])
                L = nch_half * K
                if L == 0:
                    continue
                # pad with -1: the Q7 ucode strips trailing negatives for
                # free; num_idxs_reg carries the true per-core count.  The
                # first BUFS emitted sts (the largest, one per pool buffer)
                # pad with 0 instead, fully writing the buffer: later users
                # of the same buffer have smaller chunk counts, so stripped
                # tails only ever read finite stale data.
                warm = st_rank[st] < BUFS
                idxs = np.full(L, (0 if warm else -1), np.int16)
                idxs[:len(seg_src)] = seg_src[:L]
                ncnt[0, call_of[(st, half)]] = L if warm else min(len(seg_src), L)
                co = col_off[st] + (0 if half == 0 else clo_ch[st] * (K // 16))
                gidx_view[co:co + nch_half * (K // 16), :] = \
                    idxs.reshape(nch_half * (K // 16), 16)
                # S chunks (compact order: lo chunks, then real hi chunks)
                base_ci = smat_off[st] + (0 if half == 0 else clo_ch[st])
                nreal = int(clo_ch[st] if half == 0 else chi_ch[st])
                for k in range(nreal):
                    jj = seg_j[k * K:(k + 1) * K]
                    ww = seg_w[k * K:(k + 1) * K]
                    rows = np.arange(len(jj))
                    smat[rows, (base_ci + k) * K + jj] = ww.astype(gdt)
            # ---- loop (self-edge) diagonal chunk
            ci_loop = smat_off[st] + nch_st[st] - 1
            nodes = np.arange(n0 + st * K, min(n0 + (st + 1) * K, n1))
            p = nodes - (n0 + st * K)
            smat[p, ci_loop * K + p] = loopw[nodes].astype(gdt)
            # ---- pooling matrix
            pmat[p, st * GPC + (batch[nodes] - c * GPC)] = 1.0

        gidx2 = np.tile(gidx_view.T, (8, 1))   # [128, NCOLS] replicated

        counts = np.maximum(gcnt[c * GPC:(c + 1) * GPC], 1).astype(np.float32)
        ceT = np.ascontiguousarray(
            emb[cls[c * GPC:(c + 1) * GPC]].T).astype(mmdt)   # [EH, GPC]

        m = dict(
            xlo=xlo, xhi=xhi, xown=xown, ncnt=ncnt,
            gidx=np.ascontiguousarray(gidx2),
            smat=smat,
            pmat=pmat,
            w1=W1.astype(mmdt),
            w2blk=w2blk.astype(mmdt),
            w3=w3m.astype(mmdt),
            b1=b1.reshape(1, HID).astype(mmdt),
            b2=b2.reshape(1, HID).astype(mmdt),
            b3=b3.reshape(1, 1).astype(mmdt),
            ceT=ceT,
            counts=counts.reshape(GPC, 1),
        )
        in_maps.append(m)

    prep = Prep()
    prep.static = static
    prep.in_maps = in_maps
    return prep


# ================================================================= builder
def build(static):
    import concourse.bass as bass
    from concourse import bacc, tile
    import concourse.mybir as mybir

    cfg = static["cfg"]
    N, F = cfg["N"], cfg["F"]
    XLO, K = cfg["XLO"], cfg["K"]
    NEG = cfg["NEG"]
    NQ = cfg["NQ"]
    NST, NDL, NCHMAX = static["NST"], static["NDL"], static["NCHMAX"]
    TOTCH, NCOLS = static["TOTCH"], static["NCOLS"]
    HID, EH, GPC = static["HID"], static["EH"], static["GPC"]
    NCLS = cfg["NCLS"]
    clo_ch, chi_ch = static["clo_ch"], static["chi_ch"]
    chi_eff, col_off = static["chi_eff"], static["col_off"]
    smat_off, nch_st = static["smat_off"], static["nch_st"]
    call_of, NCALLS = static["call_of"], static["NCALLS"]
    st_order = static["st_order"]
    BUFS = cfg["BUFS"]

    gdt = {"f32": mybir.dt.float32, "bf16": mybir.dt.bfloat16}[cfg["GDT"]]
    mmdt = {"f32": mybir.dt.float32, "bf16": mybir.dt.bfloat16}[cfg["MMDT"]]
    f32 = mybir.dt.float32
    AF = mybir.ActivationFunctionType

    nc = bacc.Bacc(None, target_bir_lowering=False, debug=True,
                   num_swdge_queues=NQ, dynamic_dma_scratch_size=32768)

    xlo_d = nc.declare_dram_parameter("xlo", [XLO, F], gdt, isOutput=False)
    xhi_d = nc.declare_dram_parameter("xhi", [N - XLO, F], gdt, isOutput=False)
    xown_d = nc.declare_dram_parameter("xown", [NDL, F], gdt, isOutput=False)
    gidx_d = nc.declare_dram_parameter("gidx", [128, NCOLS], mybir.dt.int16, isOutput=False)
    smat_d = nc.declare_dram_parameter("smat", [128, TOTCH * K], gdt, isOutput=False)
    pmat_d = nc.declare_dram_parameter("pmat", [128, NST * GPC], mmdt, isOutput=False)
    w1_d = nc.declare_dram_parameter("w1", [F, HID], mmdt, isOutput=False)
    w2_d = nc.declare_dram_parameter("w2blk", [128, 6 * 128], mmdt, isOutput=False)
    w3_d = nc.declare_dram_parameter("w3", [128, 2], mmdt, isOutput=False)
    b1_d = nc.declare_dram_parameter("b1", [1, HID], mmdt, isOutput=False)
    b2_d = nc.declare_dram_parameter("b2", [1, HID], mmdt, isOutput=False)
    b3_d = nc.declare_dram_parameter("b3", [1, 1], mmdt, isOutput=False)
    ceT_d = nc.declare_dram_parameter("ceT", [EH, GPC], mmdt, isOutput=False)
    cnt_d = nc.declare_dram_parameter("counts", [GPC, 1], f32, isOutput=False)
    ncnt_d = nc.declare_dram_parameter("ncnt", [1, NCALLS], mybir.dt.int32, isOutput=False)
    out_d = nc.declare_dram_parameter("out", [1, GPC], f32, isOutput=True)

    iden_np = np.eye(128, dtype=_np_dt(cfg["MMDT"]))
    iden_d = nc.inline_tensor(iden_np, name="iden")

    with tile.TileContext(nc) as tc:
        with (
            tc.tile_pool(name="const", bufs=1) as constp,
            tc.tile_pool(name="gat", bufs=BUFS) as gatp,
            tc.tile_pool(name="smp", bufs=BUFS) as smp,
            tc.tile_pool(name="work", bufs=4) as workp,
            tc.tile_pool(name="ps_agg", bufs=2, space="PSUM") as ps_agg,
            tc.tile_pool(name="ps_h", bufs=2, space="PSUM") as ps_h,
            tc.tile_pool(name="ps_t", bufs=2, space="PSUM") as ps_t,
            tc.tile_pool(name="ps_pool", bufs=1, space="PSUM") as ps_pool,
        ):
            # ---- persistent SBUF loads.  gidx is split and spread over
            # engines so the first gathers don't wait on the whole table;
            # tail-only constants go to the idle Vector/Scalar queues.
            ncnt_sb = constp.tile([1, NCALLS], mybir.dt.int32)
            nc.sync.dma_start(out=ncnt_sb[:, :], in_=ncnt_d[:, :])
            gidx_sb = constp.tile([128, NCOLS], mybir.dt.int16)
            first_sts = st_order[:4]
            for s in first_sts:
                a, b = int(col_off[s]), int(col_off[s + 1])
                if b > a:
                    nc.sync.dma_start(out=gidx_sb[:, a:b], in_=gidx_d[:, a:b])
            rest = sorted(set(range(NST)) - set(first_sts))
            runs = []
            for s in rest:
                a, b = int(col_off[s]), int(col_off[s + 1])
                if runs and runs[-1][1] == a:
                    runs[-1][1] = b
                else:
                    runs.append([a, b])
            for a, b in runs:
                if b > a:
                    nc.scalar.dma_start(out=gidx_sb[:, a:b], in_=gidx_d[:, a:b])
            w1_sb = constp.tile([F, HID], mmdt)
            nc.sync.dma_start(out=w1_sb[:, :], in_=w1_d[:, :])
            b1_sb = constp.tile([1, HID], mmdt)
            nc.sync.dma_start(out=b1_sb[:, :], in_=b1_d[:, :])
            pmat_sb = constp.tile([128, NST * GPC], mmdt)
            nc.scalar.dma_start(out=pmat_sb[:, :], in_=pmat_d[:, :])
            iden_sb = constp.tile([128, 128], mmdt)
            nc.scalar.dma_start(out=iden_sb[:, :], in_=iden_d[:, :])
            ones_sb = constp.tile([1, 128], mmdt)
            nc.vector.memset(ones_sb[:, :], 1.0)
            # tail constants on the scalar queue (sync stays free for the
            # per-supertile loop/sm loads)
            cnt_sb = constp.tile([GPC, 1], f32)
            nc.scalar.dma_start(out=cnt_sb[:, :], in_=cnt_d[:, :])
            ceT_sb = constp.tile([EH, GPC], mmdt)
            nc.scalar.dma_start(out=ceT_sb[:, :], in_=ceT_d[:, :])
            w2_sb = constp.tile([128, 6 * 128], mmdt)
            nc.scalar.dma_start(out=w2_sb[:, :], in_=w2_d[:, :])
            b2_sb = constp.tile([1, HID], mmdt)
            nc.scalar.dma_start(out=b2_sb[:, :], in_=b2_d[:, :])
            w3_sb = constp.tile([128, 2], mmdt)
            nc.scalar.dma_start(out=w3_sb[:, :], in_=w3_d[:, :])
            b3_sb = constp.tile([1, 1], mmdt)
            nc.scalar.dma_start(out=b3_sb[:, :], in_=b3_d[:, :])

            pooled_ps = ps_pool.tile([GPC, HID], f32)

            # registers for per-core true gather lengths; ncnt is stored in
            # emission order, so consecutive calls batch-load RB at a time
            RB = 8
            cnt_regs = [nc.gpsimd.alloc_register(f"gcnt{i}") for i in range(RB)]
            call_i = 0

            def next_cnt_reg():
                nonlocal call_i
                if call_i % RB == 0:
                    lo = call_i
                    hi = min(call_i + RB, NCALLS)
                    nc.gpsimd.reg_load(cnt_regs[:hi - lo],
                                       ncnt_sb[0:1, lo:hi])
                reg = cnt_regs[call_i % RB]
                call_i += 1
                return reg

            qload = [0] * NQ   # greedy least-loaded queue assignment
            for sti, st in enumerate(st_order):
                nch = int(nch_st[st])
                clo = int(clo_ch[st])
                chie = int(chi_eff[st])
                gt = gatp.tile([128, NCHMAX, F], gdt, tag="gt")
                c0 = int(col_off[st])
                if cfg.get("NO_GATHER"):
                    nc.vector.memset(gt[:, :, :], 0.125)
                else:
                    if clo > 0:
                        q = min(range(NQ), key=lambda i: qload[i])
                        qload[q] += clo * K
                        nc.gpsimd.dma_gather(
                            gt[:, 0:clo, :], xlo_d[:, :],
                            gidx_sb[:, c0:c0 + clo * (K // 16)],
                            num_idxs=clo * K, num_idxs_reg=next_cnt_reg(),
                            elem_size=F, single_packet=False,
                            queue_num=q)
                    if chie > 0:
                        q = min(range(NQ), key=lambda i: qload[i])
                        qload[q] += chie * K
                        nc.gpsimd.dma_gather(
                            gt[:, clo:clo + chie, :], xhi_d[:, :],
                            gidx_sb[:, c0 + clo * (K // 16):
                                    c0 + (clo + chie) * (K // 16)],
                            num_idxs=chie * K, num_idxs_reg=next_cnt_reg(),
                            elem_size=F, single_packet=False,
                            queue_num=q)
                # self-loop (diagonal) chunk: linear load of own dst rows
                nc.sync.dma_start(out=gt[:, NCHMAX - 1, :],
                                  in_=xown_d[st * K:(st + 1) * K, :])

                sm = smp.tile([128, NCHMAX * K], gdt, tag="sm")
                m0 = int(smat_off[st])
                nc.sync.dma_start(
                    out=sm[:, 0:nch * K],
                    in_=smat_d[:, m0 * K:(m0 + nch) * K])

                # ---- aggregation: aggT[F, dst] = sum_ci gt_ci^T-free @ S_ci
                aggT = ps_agg.tile([128, 128], f32, tag="aggT")
                for ci in range(nch):
                    gt_ci = NCHMAX - 1 if ci == nch - 1 else \
                        (ci if ci < clo else clo + (ci - clo))
                    nc.tensor.matmul(
                        aggT[:, :],
                        lhsT=gt[:, gt_ci, :],
                        rhs=sm[:, ci * K:(ci + 1) * K],
                        start=(ci == 0), stop=(ci == nch - 1))
                aggT_sb = workp.tile([128, 128], mmdt, tag="aggT_sb")
                nc.scalar.copy(out=aggT_sb[:, :], in_=aggT[:, :])

                # ---- W1 + b1, leaky relu
                h_ps = ps_h.tile([128, HID], f32, tag="h")
                nc.tensor.matmul(h_ps[:, :], lhsT=aggT_sb[:, :],
                                 rhs=w1_sb[:, :], start=True, stop=False)
                nc.tensor.matmul(h_ps[:, :], lhsT=ones_sb[:, 0:128],
                                 rhs=b1_sb[:, :], start=False, stop=True)
                hr_sb = workp.tile([128, HID], f32, tag="hr_sb")
                nc.scalar.activation(hr_sb[:, :], h_ps[:, :], AF.Relu,
                                     scale=1.0 - NEG)
                h_sb = workp.tile([128, HID], mmdt, tag="h_sb")
                nc.vector.scalar_tensor_tensor(
                    h_sb[:, :], in0=h_ps[:, :], scalar=NEG,
                    in1=hr_sb[:, :], op0=mybir.AluOpType.mult,
                    op1=mybir.AluOpType.add)
                # ---- pool accumulate (graphs of this core only)
                nc.tensor.matmul(
                    pooled_ps[:, :],
                    lhsT=pmat_sb[:, st * GPC:(st + 1) * GPC],
                    rhs=h_sb[:, :],
                    start=(sti == 0), stop=(sti == NST - 1),
                    skip_group_check=True)

            # ---------------- tail: per-core MLP on its own GPC graphs
            pooled_sb = workp.tile([GPC, HID], f32, tag="pooled")
            nc.vector.tensor_copy(out=pooled_sb[:, :], in_=pooled_ps[:, :])
            rec_sb = workp.tile([GPC, 1], f32, tag="rec")
            nc.vector.reciprocal(rec_sb[:, :], cnt_sb[:, :])
            pm_sb = workp.tile([GPC, HID], mmdt, tag="pm")
            nc.vector.tensor_scalar_mul(pm_sb[:, :], pooled_sb[:, :],
                                        rec_sb[:, :])

            # transpose pooled -> [128, GPC] halves
            zt = []
            for jj in range(HID // 128):
                tp = ps_t.tile([128, GPC], mmdt, tag="tp")
                nc.tensor.transpose(tp[:, :],
                                    pm_sb[:, jj * 128:(jj + 1) * 128],
                                    iden_sb[0:GPC, 0:GPC])
                t_sb = workp.tile([128, GPC], mmdt, tag=f"zt{jj}")
                nc.scalar.copy(out=t_sb[:, :], in_=tp[:, :])
                zt.append(t_sb)
            zt.append(ceT_sb)

            # W2 (+b2), leaky
            ones_g = workp.tile([1, GPC], mmdt, tag="onesg")
            nc.vector.memset(ones_g[:, :], 1.0)
            nk = (HID + EH) // 128
            z2 = []
            for jj in range(2):
                zp = ps_h.tile([128, GPC], f32, tag="h")
                for kk in range(nk):
                    nc.tensor.matmul(
                        zp[:, :],
                        lhsT=w2_sb[:, (kk * 2 + jj) * 128:(kk * 2 + jj + 1) * 128],
                        rhs=zt[kk][:, :], start=(kk == 0), stop=False)
                nc.tensor.matmul(zp[:, :], lhsT=b2_sb[:, jj * 128:(jj + 1) * 128],
                                 rhs=ones_g[:, :], start=False, stop=True)
                zr_sb = workp.tile([128, GPC], f32, tag="zr_sb")
                nc.scalar.activation(zr_sb[:, :], zp[:, :], AF.Relu,
                                     scale=1.0 - NEG)
                z_sb = workp.tile([128, GPC], mmdt, tag=f"z2sb{jj}")
                nc.vector.scalar_tensor_tensor(
                    z_sb[:, :], in0=zp[:, :], scalar=NEG, in1=zr_sb[:, :],
                    op0=mybir.AluOpType.mult, op1=mybir.AluOpType.add)
                z2.append(z_sb)

            # W3 (+b3)
            op = ps_h.tile([1, GPC], f32, tag="h")
            for jj in range(2):
                nc.tensor.matmul(op[:, :], lhsT=w3_sb[:, jj:jj + 1],
                                 rhs=z2[jj][:, :], start=(jj == 0), stop=False)
            nc.tensor.matmul(op[:, :], lhsT=b3_sb[:, :], rhs=ones_g[:, :],
                             start=False, stop=True)
            o_sb = workp.tile([1, GPC], f32, tag="osb")
            nc.vector.tensor_copy(out=o_sb[:, :], in_=op[:, :])
            nc.sync.dma_start(out=out_d[:, :], in_=o_sb[:, :])

    return nc


# ================================================================= runner
def _run(inputs, cfg=None, trace=False):
    from concourse.bass_utils import run_bass_kernel_spmd
    cfg = dict(CFG if cfg is None else cfg)
    for k in ("NQ", "WARM", "GRAN", "NO_GATHER", "N", "F", "HID", "G",
              "NCLS", "NCORES", "XLO", "K", "BUFS"):
        cfg[k] = int(cfg[k])
    prep = host_prep(inputs, cfg)
    nc = build(prep.static)
    nc.finalize()
    res = run_bass_kernel_spmd(
        nc, prep.in_maps, core_ids=list(range(cfg["NCORES"])), trace=trace)
    parts = [np.asarray(res.results[c]["out"], np.float32).reshape(-1)
             for c in range(cfg["NCORES"])]
    out = np.concatenate(parts).reshape(-1, 1)
    return out, res


def kernel(**inputs):
    out, _ = _run(inputs)
    return out
